# revision 1
# baseline (speedup 1.0000x reference)
"""Trainium2 Bass kernel for the GRU caption model.

Computes: h0 = feat @ W_hp.T + b_hp; 200-step GRU with constant hidden-proj
gate pre-activations; logits = outs @ W_out.T + b_out -> [B, V, T].

Strategy: every core runs the (tiny, latency-bound) GRU redundantly; the
vocab dimension of W_out is sharded 8 ways; each core emits its own
[B, 3840, T] logits slice which the host concatenates.

All on-chip compute uses a transposed [feature-on-partitions, batch-free]
layout so the recurrent state feeds the next step's matmul directly.
"""

import numpy as np
import ml_dtypes

import concourse.bass as bass
import concourse.mybir as mybir
import concourse.tile as tile
from concourse import bacc
from concourse.bass_utils import run_bass_kernel_spmd

F32 = mybir.dt.float32
F32R = mybir.dt.float32r
BF16 = mybir.dt.bfloat16
AF = mybir.ActivationFunctionType
ALU = mybir.AluOpType

VOCAB = 30522
HID = 512
FEAT = 2048
STEPS = 200
BATCH = 32
SOS = 101
NCORES = 8
P = 128
KO = HID // P          # 4 h-chunks
GM = 3 * HID // P      # 12 gate row-groups (r: 0-3, z: 4-7, n: 8-11)
KF = FEAT // P         # 16 feat chunks
VPAD = 3840            # per-core padded vocab rows = 30 * 128
MT = VPAD // P         # 30 vocab tiles per core
TBLOCKS = [(0, 64), (64, 128), (128, 200)]  # proj t-blocks

LAST_RESULTS = None  # test harness introspection
EMIT_GRU = True    # variant switch (sim experiments)
EMIT_PROJ = True   # variant switch (sim experiments)
PROJ_MODE = 2      # 0 = matmuls only, 1 = +copies, 2 = +DMA (sim experiments)


def _r(ap):
    """Reinterpret an fp32 AP as float32r for full-rate PE streaming."""
    return ap.bitcast(F32R)


def build():
    nc = bacc.Bacc("TRN2", target_bir_lowering=False, debug=False)

    featT = nc.dram_tensor("featT", [FEAT, BATCH], F32, kind="ExternalInput")
    WhpT = nc.dram_tensor("WhpT", [FEAT, HID], F32, kind="ExternalInput")
    WihT = nc.dram_tensor("WihT", [HID, 3 * HID], BF16, kind="ExternalInput")
    WhhT = nc.dram_tensor("WhhT", [HID, 3 * HID], F32, kind="ExternalInput")
    b_ih = nc.dram_tensor("b_ih", [3 * HID], F32, kind="ExternalInput")
    b_hh = nc.dram_tensor("b_hh", [3 * HID], F32, kind="ExternalInput")
    b_hp = nc.dram_tensor("b_hp", [HID], F32, kind="ExternalInput")
    x0T = nc.dram_tensor("x0T", [HID, BATCH], BF16, kind="ExternalInput")
    WoutT = nc.dram_tensor("WoutT", [HID, VPAD], F32R, kind="ExternalInput")
    b_out = nc.dram_tensor("b_out", [VPAD], F32, kind="ExternalInput")
    OUT = nc.dram_tensor("OUT", [BATCH, VPAD, STEPS], F32, kind="ExternalOutput")

    with tile.TileContext(nc) as tc:
        with (
            tc.tile_pool(name="const", bufs=1) as const,
            tc.tile_pool(name="stream", bufs=3) as stream,
            tc.tile_pool(name="step", bufs=4) as sp,
            tc.tile_pool(name="hb", bufs=4) as hb,
            tc.tile_pool(name="outp", bufs=6) as outp,
            tc.tile_pool(name="psg", bufs=3, space="PSUM") as psg,
            tc.tile_pool(name="psp", bufs=4, space="PSUM") as psp,
        ):
            # ---- constants into SBUF ----
            wih = const.tile([P, KO, GM, P], BF16, tag="wih")
            nc.sync.dma_start(
                wih[:], WihT.rearrange("(k p) (m c) -> p k m c", p=P, c=P)
            )
            featT_sb = const.tile([P, KF, BATCH], F32, tag="featsb")
            nc.sync.dma_start(featT_sb[:], featT.rearrange("(k p) b -> p k b", p=P))
            bih_sb = const.tile([P, GM], F32, tag="bih")
            nc.sync.dma_start(bih_sb[:], b_ih.rearrange("(m p) -> p m", p=P))
            bhh_sb = const.tile([P, GM], F32, tag="bhh")
            nc.sync.dma_start(bhh_sb[:], b_hh.rearrange("(m p) -> p m", p=P))
            bhp_sb = const.tile([P, KO], F32, tag="bhp")
            nc.sync.dma_start(bhp_sb[:], b_hp.rearrange("(m p) -> p m", p=P))
            bout_sb = const.tile([P, MT], F32, tag="bout")
            nc.sync.dma_start(bout_sb[:], b_out.rearrange("(m p) -> p m", p=P))

            WhpT_r = WhpT.rearrange("(k p) h -> p k h", p=P)
            WhhT_r = WhhT.rearrange("(k p) g -> p k g", p=P)
            WoutT_r = WoutT.rearrange("(k p) v -> p k v", p=P)

            # ---- h0 = feat @ W_hp.T + b_hp (fp32, exact) ----
            ps_h = psg.tile([P, GM, BATCH], F32, tag="gates")
            for ko in range(KO):
                for kf in range(KF):
                    wt = stream.tile([P, P], F32, tag="whp")
                    nc.sync.dma_start(wt[:], WhpT_r[:, kf, ko * P:(ko + 1) * P])
                    nc.tensor.matmul(
                        ps_h[:, ko, :], wt[:], featT_sb[:, kf, :],
                        start=(kf == 0), stop=(kf == KF - 1),
                    )
            h0T = const.tile([P, KO, BATCH], F32, tag="h0T")
            for ko in range(KO):
                nc.scalar.activation(
                    h0T[:, ko, :], ps_h[:, ko, :], AF.Identity,
                    bias=bhp_sb[:, ko, None], scale=1.0,
                )
            h0_half = const.tile([P, KO, BATCH], F32, tag="h0h")
            nc.scalar.mul(h0_half[:], h0T[:], 0.5)

            # ---- gh = h0 @ W_hh.T + b_hh (fp32, exact; step-invariant) ----
            ps_g = psg.tile([P, GM, BATCH], F32, tag="gates")
            for m in range(GM):
                for k in range(KO):
                    wt = stream.tile([P, P], F32, tag="whh")
                    nc.sync.dma_start(wt[:], WhhT_r[:, k, m * P:(m + 1) * P])
                    nc.tensor.matmul(
                        ps_g[:, m, :], wt[:], h0T[:, k, :],
                        start=(k == 0), stop=(k == KO - 1),
                    )
            ghT = const.tile([P, GM, BATCH], F32, tag="ghT")
            for m in range(GM):
                nc.scalar.activation(
                    ghT[:, m, :], ps_g[:, m, :], AF.Identity,
                    bias=bhh_sb[:, m, None], scale=1.0,
                )
            # C_rz = gh_rz + b_ih_rz ; hn2 = 0.5*gh_n ; E_n = hn2 + b_ih_n
            C_rz = const.tile([P, 8, BATCH], F32, tag="Crz")
            nc.vector.tensor_add(
                C_rz[:], ghT[:, 0:8, :],
                bih_sb[:, 0:8, None].to_broadcast((P, 8, BATCH)),
            )
            hn2 = const.tile([P, KO, BATCH], F32, tag="hn2")
            nc.scalar.mul(hn2[:], ghT[:, 8:12, :], 0.5)
            E_n = const.tile([P, KO, BATCH], F32, tag="En")
            nc.vector.tensor_add(
                E_n[:], hn2[:],
                bih_sb[:, 8:12, None].to_broadcast((P, KO, BATCH)),
            )

            # resT blocks: col = b*bsize + (t - t0), per h-chunk ko
            resT = []
            for j, (t0, t1) in enumerate(TBLOCKS):
                bs = t1 - t0
                rt = const.tile(
                    [P, KO, BATCH, bs], F32R, tag=f"resT{j}", name=f"resT{j}"
                )
                resT.append(rt)

            prev = hb.tile([P, KO, BATCH], BF16, tag="hb")
            nc.sync.dma_start(prev[:], x0T.rearrange("(k p) b -> p k b", p=P))

            def proj_block(j):
                t0, t1 = TBLOCKS[j]
                bs = t1 - t0
                gb = 4
                N = gb * bs
                for m in range(MT):
                    wt = stream.tile([P, KO, P], F32R, tag="wout")
                    nc.sync.dma_start(wt[:], WoutT_r[:, :, m * P:(m + 1) * P])
                    for g in range(BATCH // gb):
                        ps_full = psp.tile([P, 288], F32, tag="pp", name="pp")
                        ps = ps_full[:, :N]
                        for k in range(KO):
                            nc.tensor.matmul(
                                ps,
                                wt[:, k, :],
                                resT[j][:, k, gb * g:gb * g + gb, :],
                                start=(k == 0), stop=(k == KO - 1),
                            )
                        if PROJ_MODE == 0:
                            continue
                        ob_full = outp.tile([P, 288], F32, tag="ob", name="ob")
                        ob = ob_full[:, :N]
                        if (m + g) % 2 == 0:
                            nc.scalar.activation(
                                ob, ps, AF.Identity,
                                bias=bout_sb[:, m, None], scale=1.0,
                            )
                        else:
                            nc.vector.tensor_scalar_add(ob, ps, bout_sb[:, m, None])
                        if PROJ_MODE >= 2:
                            dst = OUT[
                                gb * g:gb * g + gb, m * P:(m + 1) * P, t0:t1
                            ].rearrange("b v t -> v b t")
                            nc.sync.dma_start(
                                dst, ob.rearrange("p (b t) -> p b t", b=gb)
                            )

            # ---- GRU steps ----
            if not EMIT_GRU:
                for j in range(len(TBLOCKS)):
                    nc.vector.memset(resT[j][:], 0.25)
                    proj_block(j)
            mm_order = [8, 9, 10, 11] + list(range(8))  # n-gates first
            for t in range(STEPS if EMIT_GRU else 0):
                ps = psg.tile([P, GM, BATCH], F32, tag="gates")
                for m in mm_order:
                    for k in range(KO):
                        nc.tensor.matmul(
                            ps[:, m, :], wih[:, k, m, :], prev[:, k, :],
                            start=(k == 0), stop=(k == KO - 1),
                        )
                s_rz = sp.tile([P, 8, BATCH], F32, tag="srz")
                nc.vector.tensor_add(s_rz[:], ps[:, 0:8, :], C_rz[:])
                t_rz = sp.tile([P, 8, BATCH], F32, tag="trz")
                nc.scalar.activation(t_rz[:], s_rz[:], AF.Tanh, scale=0.5)
                a = sp.tile([P, KO, BATCH], F32, tag="a")
                nc.vector.tensor_mul(a[:], t_rz[:, 0:4, :], hn2[:])
                sn1 = sp.tile([P, KO, BATCH], F32, tag="sn1")
                nc.vector.tensor_add(sn1[:], ps[:, 8:12, :], E_n[:])
                sn2 = sp.tile([P, KO, BATCH], F32, tag="sn2")
                nc.vector.tensor_add(sn2[:], sn1[:], a[:])
                n = sp.tile([P, KO, BATCH], F32, tag="n")
                nc.scalar.activation(n[:], sn2[:], AF.Tanh, scale=1.0)
                q = sp.tile([P, KO, BATCH], F32, tag="q")
                nc.vector.tensor_sub(q[:], h0T[:], n[:])
                w2 = sp.tile([P, KO, BATCH], F32, tag="w2")
                nc.vector.scalar_tensor_tensor(
                    w2[:], t_rz[:, 4:8, :], 0.5, q[:], ALU.mult, ALU.mult
                )
                p2 = sp.tile([P, KO, BATCH], F32, tag="p2")
                nc.vector.scalar_tensor_tensor(
                    p2[:], n[:], 0.5, h0_half[:], ALU.mult, ALU.add
                )
                nxt = hb.tile([P, KO, BATCH], BF16, tag="hb")
                nc.vector.tensor_add(nxt[:], w2[:], p2[:])
                j = next(i for i, (a, b) in enumerate(TBLOCKS) if a <= t < b)
                t0 = TBLOCKS[j][0]
                nc.gpsimd.tensor_add(resT[j][:, :, :, t - t0], w2[:], p2[:])
                prev = nxt
                if t == TBLOCKS[j][1] - 1 and EMIT_PROJ:
                    proj_block(j)

    nc.compile()
    return nc


def _shard_inputs(feat, W_hp, b_hp, W_ih, W_hh, b_ih, b_hh, embed, W_out, b_out):
    bf = ml_dtypes.bfloat16
    featT = np.ascontiguousarray(feat.T, dtype=np.float32)
    WhpT = np.ascontiguousarray(W_hp.T, dtype=np.float32)
    WihT = np.ascontiguousarray(W_ih.T).astype(bf)
    WhhT = np.ascontiguousarray(W_hh.T, dtype=np.float32)
    x0T = np.ascontiguousarray(
        np.repeat(np.asarray(embed)[SOS][:, None], BATCH, axis=1)
    ).astype(bf)
    Wo = np.zeros((NCORES * VPAD, HID), np.float32)
    Wo[:VOCAB] = W_out
    bo = np.zeros((NCORES * VPAD,), np.float32)
    bo[:VOCAB] = b_out
    common = dict(
        featT=featT, WhpT=WhpT, WihT=WihT, WhhT=WhhT,
        b_ih=np.asarray(b_ih, np.float32), b_hh=np.asarray(b_hh, np.float32),
        b_hp=np.asarray(b_hp, np.float32), x0T=x0T,
    )
    in_maps = []
    for c in range(NCORES):
        sl = slice(c * VPAD, (c + 1) * VPAD)
        m = dict(common)
        m["WoutT"] = np.ascontiguousarray(Wo[sl].T)
        m["b_out"] = bo[sl].copy()
        in_maps.append(m)
    return in_maps


def kernel(**inputs):
    global LAST_RESULTS
    args = {k: np.asarray(v) for k, v in inputs.items()}
    in_maps = _shard_inputs(
        args["feat"], args["W_hp"], args["b_hp"], args["W_ih"], args["W_hh"],
        args["b_ih"], args["b_hh"], args["embed"], args["W_out"], args["b_out"],
    )
    nc = build()
    res = run_bass_kernel_spmd(nc, in_maps, core_ids=list(range(NCORES)))
    LAST_RESULTS = res
    out = np.concatenate([r["OUT"] for r in res.results], axis=1)[:, :VOCAB, :]
    return np.ascontiguousarray(out, dtype=np.float32)



# revision 7
# speedup vs baseline: 1.6748x; 1.6748x over previous
"""Trainium2 Bass kernel for the GRU caption model.

Computes: h0 = feat @ W_hp.T + b_hp; 200-step GRU with constant hidden-proj
gate pre-activations; logits = outs @ W_out.T (+ b_out on host) -> [B, V, T].

Strategy: every core runs the (tiny, latency-bound) GRU redundantly; the
vocab dimension of W_out is sharded 8 ways.  The projection uses the GRU
state tiles as the *stationary* matmul operand ([128 h, 128 (t,b)] chunks)
and streams W_out columns, so each 4-timestep "granule" yields a
[128 (t,b), 3840 v] fp16 tile that leaves in a single large DMA.  Proj
matmuls are interleaved two units per GRU step to keep the PE continuously
busy (full p-state).  Gate constants (W_hh @ h0 + biases) are accumulated
into the gates PSUM through a small identity matmul, keeping the serial
per-step chain to: mm -> tanh(r) -> mul/add -> tanh(n) -> mul/add -> mm.
"""

import numpy as np
import ml_dtypes

import concourse.bass as bass
import concourse.mybir as mybir
import concourse.tile as tile
from concourse import bacc
from concourse.bass_utils import run_bass_kernel_spmd

F32 = mybir.dt.float32
BF16 = mybir.dt.bfloat16
FP16 = mybir.dt.float16
AF = mybir.ActivationFunctionType
ALU = mybir.AluOpType

VOCAB = 30522
HID = 512
FEAT = 2048
STEPS = 200
BATCH = 32
SOS = 101
NCORES = 8
P = 128
KO = HID // P            # 4 h-chunks
GM = 3 * HID // P        # 12 gate row-groups (r: 0-3, z: 4-7, n: 8-11)
KF = FEAT // P           # 16 feat chunks
VPAD = 3840              # per-core padded vocab rows = 30 * 128
GR = 4                   # granule = 4 timesteps -> 128 (t,b) columns
NGRAN = STEPS // GR      # 50
VC = 480                 # proj v-chunk columns (psum bank holds <=512 f32)
NVC = VPAD // VC         # 8 units per granule

LAST_RESULTS = None  # test harness introspection


def build():
    nc = bacc.Bacc("TRN2", target_bir_lowering=False, debug=False)

    featT = nc.dram_tensor("featT", [FEAT, BATCH], F32, kind="ExternalInput")
    WhpT = nc.dram_tensor("WhpT", [FEAT, HID], F32, kind="ExternalInput")
    WhhT = nc.dram_tensor("WhhT", [HID, 3 * HID], F32, kind="ExternalInput")
    WihT = nc.dram_tensor("WihT", [HID, 3 * HID], BF16, kind="ExternalInput")
    x0T = nc.dram_tensor("x0T", [HID, BATCH], BF16, kind="ExternalInput")
    WoutT = nc.dram_tensor("WoutT", [HID, VPAD], BF16, kind="ExternalInput")
    b_hp = nc.dram_tensor("b_hp", [HID], F32, kind="ExternalInput")
    bsum_rz = nc.dram_tensor("bsum_rz", [2 * HID], F32, kind="ExternalInput")
    bmix_n = nc.dram_tensor("bmix_n", [HID], F32, kind="ExternalInput")
    bhhn_half = nc.dram_tensor("bhhn_half", [HID], F32, kind="ExternalInput")
    I32 = nc.dram_tensor("I32", [32, 32], BF16, kind="ExternalInput")
    I128 = nc.dram_tensor("I128", [P, P], F32, kind="ExternalInput")
    # row (t*BATCH + b) holds logits[b, :, t] for this core's vocab slice
    OUT = nc.dram_tensor("OUT", [STEPS * BATCH, VPAD], FP16, kind="ExternalOutput")

    with tile.TileContext(nc) as tc:
        with (
            tc.tile_pool(name="const", bufs=1) as const,
            tc.tile_pool(name="stage", bufs=3) as stagep,
            tc.tile_pool(name="step", bufs=3) as sp,
            tc.tile_pool(name="psg", bufs=2, space="PSUM") as psg,
            tc.tile_pool(name="psp", bufs=4, space="PSUM") as psp,
            tc.tile_pool(name="pst", bufs=1, space="PSUM") as pst,
        ):
            # ---- constants into SBUF ----
            wih = const.tile([P, KO, GM, P], BF16, tag="wih")
            nc.sync.dma_start(
                wih[:], WihT.rearrange("(k p) (m c) -> p k m c", p=P, c=P)
            )
            wout = const.tile([P, KO, VPAD], BF16, tag="wout")
            nc.sync.dma_start(wout[:], WoutT.rearrange("(k p) v -> p k v", p=P))
            whp_sb = const.tile([P, KF, HID], F32, tag="whp")
            nc.sync.dma_start(whp_sb[:], WhpT.rearrange("(k p) h -> p k h", p=P))
            whh_sb = const.tile([P, KO, 3 * HID], F32, tag="whh")
            nc.sync.dma_start(whh_sb[:], WhhT.rearrange("(k p) g -> p k g", p=P))
            featT_sb = const.tile([P, KF, BATCH], F32, tag="featsb")
            nc.sync.dma_start(featT_sb[:], featT.rearrange("(k p) b -> p k b", p=P))
            bhp_sb = const.tile([P, KO], F32, tag="bhp")
            nc.sync.dma_start(bhp_sb[:], b_hp.rearrange("(m p) -> p m", p=P))
            bsrz_sb = const.tile([P, 8], F32, tag="bsrz")
            nc.sync.dma_start(bsrz_sb[:], bsum_rz.rearrange("(m p) -> p m", p=P))
            bmixn_sb = const.tile([P, KO], F32, tag="bmixn")
            nc.sync.dma_start(bmixn_sb[:], bmix_n.rearrange("(m p) -> p m", p=P))
            bhhnh_sb = const.tile([P, KO], F32, tag="bhhnh")
            nc.sync.dma_start(bhhnh_sb[:], bhhn_half.rearrange("(m p) -> p m", p=P))
            i32_sb = const.tile([32, 32], BF16, tag="i32")
            nc.sync.dma_start(i32_sb[:], I32[:, :])
            i128_sb = const.tile([P, P], F32, tag="i128")
            nc.sync.dma_start(i128_sb[:], I128[:, :])
            x0_sb = const.tile([P, KO, BATCH], BF16, tag="x0")
            nc.sync.dma_start(x0_sb[:], x0T.rearrange("(k p) b -> p k b", p=P))

            # resT[p, k, t, b] = h_{t+1}[k*128+p, b]; layout puts (t, b) last
            # so a 4-step granule slice is a contiguous 128-column stationary
            # operand for the projection matmuls.
            resT = const.tile([P, KO, STEPS, BATCH], BF16, tag="resT")

            # ---- h0 = feat @ W_hp.T + b_hp (fp32, exact) ----
            ps_h = psg.tile([P, GM, BATCH], F32, tag="gates")
            for ko in range(KO):
                for kf in range(KF):
                    nc.tensor.matmul(
                        ps_h[:, ko, :],
                        whp_sb[:, kf, ko * P:(ko + 1) * P],
                        featT_sb[:, kf, :],
                        start=(kf == 0), stop=(kf == KF - 1),
                    )
            h0T = const.tile([P, KO, BATCH], F32, tag="h0T")
            for ko in range(KO):
                nc.scalar.activation(
                    h0T[:, ko, :], ps_h[:, ko, :], AF.Identity,
                    bias=bhp_sb[:, ko, None], scale=1.0,
                )
            h0h = const.tile([P, KO, BATCH], BF16, tag="h0h")
            nc.scalar.mul(h0h[:], h0T[:], 0.5)

            # ---- gate constants G0 (fp32 psum -> transposed bf16) ----
            # rz rows: G0 = W_hh@h0 + b_hh + b_ih
            # n rows:  G0 = 0.5*(W_hh@h0 + b_hh) + b_ih   (E_n form)
            ps_g = psg.tile([P, GM, BATCH], F32, tag="gates")
            for m in range(GM):
                for k in range(KO):
                    nc.tensor.matmul(
                        ps_g[:, m, :],
                        whh_sb[:, k, m * P:(m + 1) * P],
                        h0T[:, k, :],
                        start=(k == 0), stop=(k == KO - 1),
                    )
            G0 = const.tile([P, GM, BATCH], F32, tag="G0")
            nc.vector.tensor_add(
                G0[:, 0:8, :], ps_g[:, 0:8, :],
                bsrz_sb[:, :, None].to_broadcast((P, 8, BATCH)),
            )
            nc.vector.scalar_tensor_tensor(
                G0[:, 8:12, :], ps_g[:, 8:12, :], 0.5,
                bmixn_sb[:, :, None].to_broadcast((P, KO, BATCH)),
                ALU.mult, ALU.add,
            )
            # hn2 = 0.5*(W_hh@h0 + b_hh)_n
            hn2 = const.tile([P, KO, BATCH], BF16, tag="hn2")
            nc.vector.scalar_tensor_tensor(
                hn2[:], ps_g[:, 8:12, :], 0.5,
                bhhnh_sb[:, :, None].to_broadcast((P, KO, BATCH)),
                ALU.mult, ALU.add,
            )
            # G0T[b, m*128+p] = G0[p, m, b], bf16 for the per-step const matmul
            G0T = const.tile([32, GM * P], BF16, tag="G0T")
            for m in range(GM):
                tp = pst.tile([32, P], F32, tag="tp")
                nc.tensor.transpose(tp[:], G0[:, m, :], i128_sb[:])
                nc.scalar.copy(G0T[:, m * P:(m + 1) * P], tp[:])

            # ---- per-granule projection state ----
            stage_tiles = {}

            def emit_proj_mm(g, u):
                if u == 0:
                    stage_tiles[g] = stagep.tile(
                        [P, VPAD], FP16, tag="stage", name=f"stage{g}"
                    )
                pp = psp.tile([P, VC], F32, tag="pp", name=f"pp{g}_{u}")
                for k in range(KO):
                    nc.tensor.matmul(
                        pp[:],
                        resT[:, k, g * GR:(g + 1) * GR, :],
                        wout[:, k, u * VC:(u + 1) * VC],
                        start=(k == 0), stop=(k == KO - 1),
                    )
                return pp

            def emit_proj_tail(g, u, pp):
                st = stage_tiles[g]
                if u % 2 == 0:
                    nc.scalar.copy(st[:, u * VC:(u + 1) * VC], pp[:])
                else:
                    nc.vector.tensor_scalar_add(st[:, u * VC:(u + 1) * VC], pp[:], 0.0)
                if u == NVC - 1:
                    nc.sync.dma_start(OUT[g * P:(g + 1) * P, :], st[:])
                    del stage_tiles[g]

            # ---- GRU steps ----
            for t in range(STEPS):
                ps = psg.tile([P, GM, BATCH], F32, tag="gates")
                for m in range(GM):
                    nc.tensor.matmul(
                        ps[:, m, :], G0T[:, m * P:(m + 1) * P], i32_sb[:],
                        start=True, stop=False,
                    )
                    for k in range(KO):
                        rhs = x0_sb[:, k, :] if t == 0 else resT[:, k, t - 1, :]
                        nc.tensor.matmul(
                            ps[:, m, :], wih[:, k, m, :], rhs,
                            start=False, stop=(k == KO - 1),
                        )
                # two projection units (granule g = t//4 - 1) interleave here
                # to keep the PE stream continuously busy
                pps = []
                g = t // GR - 1
                if g >= 0:
                    for u in (2 * (t % GR), 2 * (t % GR) + 1):
                        pps.append((g, u, emit_proj_mm(g, u)))

                tr = sp.tile([P, KO, BATCH], BF16, tag="tr")
                nc.scalar.activation(tr[:], ps[:, 0:4, :], AF.Tanh, scale=0.5)
                tz = sp.tile([P, KO, BATCH], BF16, tag="tz")
                nc.scalar.activation(tz[:], ps[:, 4:8, :], AF.Tanh, scale=0.5)
                a = sp.tile([P, KO, BATCH], BF16, tag="a")
                nc.vector.tensor_mul(a[:], tr[:], hn2[:])
                sn = sp.tile([P, KO, BATCH], BF16, tag="sn")
                nc.vector.tensor_add(sn[:], ps[:, 8:12, :], a[:])
                n_ = sp.tile([P, KO, BATCH], BF16, tag="n")
                nc.scalar.activation(n_[:], sn[:], AF.Tanh, scale=1.0)
                # d = 0.5 - 0.5*tz ; c1 = h0h*(1+tz) = h0h + h0h*tz
                d = sp.tile([P, KO, BATCH], BF16, tag="d")
                nc.vector.tensor_scalar(d[:], tz[:], -0.5, 0.5, ALU.mult, ALU.add)
                u_ = sp.tile([P, KO, BATCH], BF16, tag="u")
                nc.vector.tensor_mul(u_[:], tz[:], h0h[:])
                c1 = sp.tile([P, KO, BATCH], BF16, tag="c1")
                nc.vector.tensor_add(c1[:], u_[:], h0h[:])
                e = sp.tile([P, KO, BATCH], BF16, tag="e")
                nc.vector.tensor_mul(e[:], n_[:], d[:])
                # h' = e + c1, written straight into the res history
                nc.vector.tensor_add(resT[:, :, t, :], e[:], c1[:])

                for g, u, pp in pps:
                    emit_proj_tail(g, u, pp)

            # ---- drain the last granule's projection ----
            g = NGRAN - 1
            for u in range(NVC):
                pp = emit_proj_mm(g, u)
                emit_proj_tail(g, u, pp)

    nc.compile()
    return nc


def _shard_inputs(feat, W_hp, b_hp, W_ih, W_hh, b_ih, b_hh, embed, W_out, b_out):
    bf = ml_dtypes.bfloat16
    featT = np.ascontiguousarray(np.asarray(feat).T, dtype=np.float32)
    WhpT = np.ascontiguousarray(np.asarray(W_hp).T, dtype=np.float32)
    WhhT = np.ascontiguousarray(np.asarray(W_hh).T, dtype=np.float32)
    WihT = np.ascontiguousarray(np.asarray(W_ih).T).astype(bf)
    x0T = np.ascontiguousarray(
        np.repeat(np.asarray(embed)[SOS][:, None], BATCH, axis=1)
    ).astype(bf)
    b_ih = np.asarray(b_ih, np.float32)
    b_hh = np.asarray(b_hh, np.float32)
    bsum_rz = (b_hh + b_ih)[:2 * HID].copy()
    bmix_n = (0.5 * b_hh + b_ih)[2 * HID:].copy()
    bhhn_half = (0.5 * b_hh)[2 * HID:].copy()
    Wo = np.zeros((NCORES * VPAD, HID), np.float32)
    Wo[:VOCAB] = np.asarray(W_out)
    common = dict(
        featT=featT, WhpT=WhpT, WhhT=WhhT, WihT=WihT, x0T=x0T,
        b_hp=np.asarray(b_hp, np.float32),
        bsum_rz=bsum_rz, bmix_n=bmix_n, bhhn_half=bhhn_half,
        I32=np.eye(32, dtype=np.float32).astype(bf),
        I128=np.eye(P, dtype=np.float32),
    )
    in_maps = []
    for c in range(NCORES):
        sl = slice(c * VPAD, (c + 1) * VPAD)
        m = dict(common)
        m["WoutT"] = np.ascontiguousarray(Wo[sl].T).astype(bf)
        in_maps.append(m)
    return in_maps


def kernel(**inputs):
    global LAST_RESULTS
    args = {k: np.asarray(v) for k, v in inputs.items()}
    in_maps = _shard_inputs(
        args["feat"], args["W_hp"], args["b_hp"], args["W_ih"], args["W_hh"],
        args["b_ih"], args["b_hh"], args["embed"], args["W_out"], args["b_out"],
    )
    nc = build()
    res = run_bass_kernel_spmd(nc, in_maps, core_ids=list(range(NCORES)))
    LAST_RESULTS = res
    full = np.empty((BATCH, VOCAB, STEPS), np.float32)
    for c in range(NCORES):
        v0 = c * VPAD
        nv = min(VPAD, VOCAB - v0)
        if nv <= 0:
            break
        # OUT is [(T*B), VPAD] fp16, row t*B + b
        o = np.asarray(res.results[c]["OUT"], dtype=np.float32)
        o = o.reshape(STEPS, BATCH, VPAD)
        full[:, v0:v0 + nv, :] = o[:, :, :nv].transpose(1, 2, 0)
    b_out = np.asarray(args["b_out"], np.float32)
    if np.any(b_out):
        full += b_out[None, :, None]
    return np.ascontiguousarray(full, dtype=np.float32)


# revision 8
# speedup vs baseline: 1.9141x; 1.1429x over previous
"""Trainium2 Bass kernel for the GRU caption model.

Computes: h0 = feat @ W_hp.T + b_hp; 200-step GRU with constant hidden-proj
gate pre-activations; logits = outs @ W_out.T (+ b_out on host) -> [B, V, T].

Strategy: every core runs the (tiny, latency-bound) GRU redundantly; the
vocab dimension of W_out is sharded 8 ways.  The projection uses the GRU
state tiles as the *stationary* matmul operand ([128 h, 128 (t,b)] chunks)
and streams W_out columns, so each 4-timestep "granule" yields a
[128 (t,b), 3840 v] fp16 tile that leaves in a single large DMA.  Proj
matmuls are interleaved two units per GRU step to keep the PE continuously
busy (full p-state).  Gate constants (W_hh @ h0 + biases) are accumulated
into the gates PSUM through a small identity matmul, keeping the serial
per-step chain to: mm -> tanh(r) -> mul/add -> tanh(n) -> mul/add -> mm.
"""

import numpy as np
import ml_dtypes

import concourse.bass as bass
import concourse.mybir as mybir
import concourse.tile as tile
from concourse import bacc
from concourse.bass_utils import run_bass_kernel_spmd

F32 = mybir.dt.float32
BF16 = mybir.dt.bfloat16
FP16 = mybir.dt.float16
AF = mybir.ActivationFunctionType
ALU = mybir.AluOpType

VOCAB = 30522
HID = 512
FEAT = 2048
STEPS = 200
BATCH = 32
SOS = 101
NCORES = 8
P = 128
KO = HID // P            # 4 h-chunks
GM = 3 * HID // P        # 12 gate row-groups (r: 0-3, z: 4-7, n: 8-11)
KF = FEAT // P           # 16 feat chunks
VPAD = 3840              # per-core padded vocab rows = 30 * 128
GR = 4                   # granule = 4 timesteps -> 128 (t,b) columns
NGRAN = STEPS // GR      # 50
VC = 480                 # proj v-chunk columns (psum bank holds <=512 f32)
NVC = VPAD // VC         # 8 units per granule

LAST_RESULTS = None  # test harness introspection


def build():
    nc = bacc.Bacc("TRN2", target_bir_lowering=False, debug=False)

    featT = nc.dram_tensor("featT", [FEAT, BATCH], F32, kind="ExternalInput")
    WhpT = nc.dram_tensor("WhpT", [FEAT, HID], F32, kind="ExternalInput")
    WhhT = nc.dram_tensor("WhhT", [HID, 3 * HID], F32, kind="ExternalInput")
    WihT = nc.dram_tensor("WihT", [HID, 3 * HID], BF16, kind="ExternalInput")
    x0T = nc.dram_tensor("x0T", [HID, BATCH], BF16, kind="ExternalInput")
    WoutT = nc.dram_tensor("WoutT", [HID, VPAD], BF16, kind="ExternalInput")
    b_hp = nc.dram_tensor("b_hp", [HID], F32, kind="ExternalInput")
    bsum_rz = nc.dram_tensor("bsum_rz", [2 * HID], F32, kind="ExternalInput")
    bmix_n = nc.dram_tensor("bmix_n", [HID], F32, kind="ExternalInput")
    bhhn_half = nc.dram_tensor("bhhn_half", [HID], F32, kind="ExternalInput")
    I32 = nc.dram_tensor("I32", [32, 32], BF16, kind="ExternalInput")
    I128 = nc.dram_tensor("I128", [P, P], F32, kind="ExternalInput")
    # row (t*BATCH + b) holds logits[b, :, t] for this core's vocab slice
    OUT = nc.dram_tensor("OUT", [STEPS * BATCH, VPAD], FP16, kind="ExternalOutput")

    with tile.TileContext(nc) as tc:
        with (
            tc.tile_pool(name="const", bufs=1) as const,
            tc.tile_pool(name="stage", bufs=3) as stagep,
            tc.tile_pool(name="step", bufs=3) as sp,
            tc.tile_pool(name="psg", bufs=2, space="PSUM") as psg,
            tc.tile_pool(name="psp", bufs=4, space="PSUM") as psp,
            tc.tile_pool(name="pst", bufs=1, space="PSUM") as pst,
        ):
            # ---- constants into SBUF ----
            wih = const.tile([P, KO, GM, P], BF16, tag="wih")
            nc.sync.dma_start(
                wih[:], WihT.rearrange("(k p) (m c) -> p k m c", p=P, c=P)
            )
            wout = const.tile([P, KO, VPAD], BF16, tag="wout")
            nc.sync.dma_start(wout[:], WoutT.rearrange("(k p) v -> p k v", p=P))
            whp_sb = const.tile([P, KF, HID], F32, tag="whp")
            nc.sync.dma_start(whp_sb[:], WhpT.rearrange("(k p) h -> p k h", p=P))
            whh_sb = const.tile([P, KO, 3 * HID], F32, tag="whh")
            nc.sync.dma_start(whh_sb[:], WhhT.rearrange("(k p) g -> p k g", p=P))
            featT_sb = const.tile([P, KF, BATCH], F32, tag="featsb")
            nc.sync.dma_start(featT_sb[:], featT.rearrange("(k p) b -> p k b", p=P))
            bhp_sb = const.tile([P, KO], F32, tag="bhp")
            nc.sync.dma_start(bhp_sb[:], b_hp.rearrange("(m p) -> p m", p=P))
            bsrz_sb = const.tile([P, 8], F32, tag="bsrz")
            nc.sync.dma_start(bsrz_sb[:], bsum_rz.rearrange("(m p) -> p m", p=P))
            bmixn_sb = const.tile([P, KO], F32, tag="bmixn")
            nc.sync.dma_start(bmixn_sb[:], bmix_n.rearrange("(m p) -> p m", p=P))
            bhhnh_sb = const.tile([P, KO], F32, tag="bhhnh")
            nc.sync.dma_start(bhhnh_sb[:], bhhn_half.rearrange("(m p) -> p m", p=P))
            i32_sb = const.tile([32, 32], BF16, tag="i32")
            nc.sync.dma_start(i32_sb[:], I32[:, :])
            i128_sb = const.tile([P, P], F32, tag="i128")
            nc.sync.dma_start(i128_sb[:], I128[:, :])
            x0_sb = const.tile([P, KO, BATCH], BF16, tag="x0")
            nc.sync.dma_start(x0_sb[:], x0T.rearrange("(k p) b -> p k b", p=P))

            # resT[p, k, t, b] = h_{t+1}[k*128+p, b]; layout puts (t, b) last
            # so a 4-step granule slice is a contiguous 128-column stationary
            # operand for the projection matmuls.
            resT = const.tile([P, KO, STEPS, BATCH], BF16, tag="resT")

            # ---- h0 = feat @ W_hp.T + b_hp (fp32, exact) ----
            ps_h = psg.tile([P, GM, BATCH], F32, tag="gates")
            for ko in range(KO):
                for kf in range(KF):
                    nc.tensor.matmul(
                        ps_h[:, ko, :],
                        whp_sb[:, kf, ko * P:(ko + 1) * P],
                        featT_sb[:, kf, :],
                        start=(kf == 0), stop=(kf == KF - 1),
                    )
            h0T = const.tile([P, KO, BATCH], F32, tag="h0T")
            for ko in range(KO):
                nc.scalar.activation(
                    h0T[:, ko, :], ps_h[:, ko, :], AF.Identity,
                    bias=bhp_sb[:, ko, None], scale=1.0,
                )
            h0h = const.tile([P, KO, BATCH], BF16, tag="h0h")
            nc.scalar.mul(h0h[:], h0T[:], 0.5)

            # ---- gate constants G0 (fp32 psum -> transposed bf16) ----
            # rz rows: G0 = W_hh@h0 + b_hh + b_ih
            # n rows:  G0 = 0.5*(W_hh@h0 + b_hh) + b_ih   (E_n form)
            ps_g = psg.tile([P, GM, BATCH], F32, tag="gates")
            for m in range(GM):
                for k in range(KO):
                    nc.tensor.matmul(
                        ps_g[:, m, :],
                        whh_sb[:, k, m * P:(m + 1) * P],
                        h0T[:, k, :],
                        start=(k == 0), stop=(k == KO - 1),
                    )
            G0 = const.tile([P, GM, BATCH], F32, tag="G0")
            nc.vector.tensor_add(
                G0[:, 0:8, :], ps_g[:, 0:8, :],
                bsrz_sb[:, :, None].to_broadcast((P, 8, BATCH)),
            )
            nc.vector.scalar_tensor_tensor(
                G0[:, 8:12, :], ps_g[:, 8:12, :], 0.5,
                bmixn_sb[:, :, None].to_broadcast((P, KO, BATCH)),
                ALU.mult, ALU.add,
            )
            # hn2 = 0.5*(W_hh@h0 + b_hh)_n
            hn2 = const.tile([P, KO, BATCH], BF16, tag="hn2")
            nc.vector.scalar_tensor_tensor(
                hn2[:], ps_g[:, 8:12, :], 0.5,
                bhhnh_sb[:, :, None].to_broadcast((P, KO, BATCH)),
                ALU.mult, ALU.add,
            )
            # G0T[b, m*128+p] = G0[p, m, b], bf16 for the per-step const matmul
            G0T = const.tile([32, GM * P], BF16, tag="G0T")
            for m in range(GM):
                tp = pst.tile([32, P], F32, tag="tp")
                nc.tensor.transpose(tp[:], G0[:, m, :], i128_sb[:])
                nc.scalar.copy(G0T[:, m * P:(m + 1) * P], tp[:])

            # ---- per-granule projection state ----
            stage_tiles = {}

            def emit_proj_mm(g, u):
                if u == 0:
                    stage_tiles[g] = stagep.tile(
                        [P, VPAD], FP16, tag="stage", name=f"stage{g}"
                    )
                pp = psp.tile([P, VC], F32, tag="pp", name=f"pp{g}_{u}")
                for k in range(KO):
                    nc.tensor.matmul(
                        pp[:],
                        resT[:, k, g * GR:(g + 1) * GR, :],
                        wout[:, k, u * VC:(u + 1) * VC],
                        start=(k == 0), stop=(k == KO - 1),
                    )
                return pp

            def emit_proj_tail(g, u, pp):
                # Pool engine is otherwise idle; keeping the PSUM->fp16 drains
                # off Act/DVE protects the serial GRU chain from head-of-line
                # blocking in those engine queues.
                st = stage_tiles[g]
                nc.gpsimd.tensor_scalar_add(st[:, u * VC:(u + 1) * VC], pp[:], 0.0)
                if u == NVC - 1:
                    nc.sync.dma_start(OUT[g * P:(g + 1) * P, :], st[:])
                    del stage_tiles[g]

            # ---- GRU steps ----
            for t in range(STEPS):
                ps = psg.tile([P, GM, BATCH], F32, tag="gates")
                for m in range(GM):
                    nc.tensor.matmul(
                        ps[:, m, :], G0T[:, m * P:(m + 1) * P], i32_sb[:],
                        start=True, stop=False,
                    )
                    for k in range(KO):
                        rhs = x0_sb[:, k, :] if t == 0 else resT[:, k, t - 1, :]
                        nc.tensor.matmul(
                            ps[:, m, :], wih[:, k, m, :], rhs,
                            start=False, stop=(k == KO - 1),
                        )
                # two projection units (granule g = t//4 - 1) interleave here
                # to keep the PE stream continuously busy
                pps = []
                g = t // GR - 1
                if g >= 0:
                    for u in (2 * (t % GR), 2 * (t % GR) + 1):
                        pps.append((g, u, emit_proj_mm(g, u)))

                tr = sp.tile([P, KO, BATCH], BF16, tag="tr")
                nc.scalar.activation(tr[:], ps[:, 0:4, :], AF.Tanh, scale=0.5)
                tz = sp.tile([P, KO, BATCH], BF16, tag="tz")
                nc.scalar.activation(tz[:], ps[:, 4:8, :], AF.Tanh, scale=0.5)
                a = sp.tile([P, KO, BATCH], BF16, tag="a")
                nc.vector.tensor_mul(a[:], tr[:], hn2[:])
                sn = sp.tile([P, KO, BATCH], BF16, tag="sn")
                nc.vector.tensor_add(sn[:], ps[:, 8:12, :], a[:])
                n_ = sp.tile([P, KO, BATCH], BF16, tag="n")
                nc.scalar.activation(n_[:], sn[:], AF.Tanh, scale=1.0)
                # d = 0.5 - 0.5*tz ; c1 = h0h*(1+tz) = h0h + h0h*tz
                d = sp.tile([P, KO, BATCH], BF16, tag="d")
                nc.vector.tensor_scalar(d[:], tz[:], -0.5, 0.5, ALU.mult, ALU.add)
                u_ = sp.tile([P, KO, BATCH], BF16, tag="u")
                nc.vector.tensor_mul(u_[:], tz[:], h0h[:])
                c1 = sp.tile([P, KO, BATCH], BF16, tag="c1")
                nc.vector.tensor_add(c1[:], u_[:], h0h[:])
                e = sp.tile([P, KO, BATCH], BF16, tag="e")
                nc.vector.tensor_mul(e[:], n_[:], d[:])
                # h' = e + c1, written straight into the res history
                nc.vector.tensor_add(resT[:, :, t, :], e[:], c1[:])

                for g, u, pp in pps:
                    emit_proj_tail(g, u, pp)

            # ---- drain the last granule's projection ----
            g = NGRAN - 1
            for u in range(NVC):
                pp = emit_proj_mm(g, u)
                emit_proj_tail(g, u, pp)

    nc.compile()
    return nc


def _shard_inputs(feat, W_hp, b_hp, W_ih, W_hh, b_ih, b_hh, embed, W_out, b_out):
    bf = ml_dtypes.bfloat16
    featT = np.ascontiguousarray(np.asarray(feat).T, dtype=np.float32)
    WhpT = np.ascontiguousarray(np.asarray(W_hp).T, dtype=np.float32)
    WhhT = np.ascontiguousarray(np.asarray(W_hh).T, dtype=np.float32)
    WihT = np.ascontiguousarray(np.asarray(W_ih).T).astype(bf)
    x0T = np.ascontiguousarray(
        np.repeat(np.asarray(embed)[SOS][:, None], BATCH, axis=1)
    ).astype(bf)
    b_ih = np.asarray(b_ih, np.float32)
    b_hh = np.asarray(b_hh, np.float32)
    bsum_rz = (b_hh + b_ih)[:2 * HID].copy()
    bmix_n = (0.5 * b_hh + b_ih)[2 * HID:].copy()
    bhhn_half = (0.5 * b_hh)[2 * HID:].copy()
    Wo = np.zeros((NCORES * VPAD, HID), np.float32)
    Wo[:VOCAB] = np.asarray(W_out)
    common = dict(
        featT=featT, WhpT=WhpT, WhhT=WhhT, WihT=WihT, x0T=x0T,
        b_hp=np.asarray(b_hp, np.float32),
        bsum_rz=bsum_rz, bmix_n=bmix_n, bhhn_half=bhhn_half,
        I32=np.eye(32, dtype=np.float32).astype(bf),
        I128=np.eye(P, dtype=np.float32),
    )
    in_maps = []
    for c in range(NCORES):
        sl = slice(c * VPAD, (c + 1) * VPAD)
        m = dict(common)
        m["WoutT"] = np.ascontiguousarray(Wo[sl].T).astype(bf)
        in_maps.append(m)
    return in_maps


def kernel(**inputs):
    global LAST_RESULTS
    args = {k: np.asarray(v) for k, v in inputs.items()}
    in_maps = _shard_inputs(
        args["feat"], args["W_hp"], args["b_hp"], args["W_ih"], args["W_hh"],
        args["b_ih"], args["b_hh"], args["embed"], args["W_out"], args["b_out"],
    )
    nc = build()
    res = run_bass_kernel_spmd(nc, in_maps, core_ids=list(range(NCORES)))
    LAST_RESULTS = res
    full = np.empty((BATCH, VOCAB, STEPS), np.float32)
    for c in range(NCORES):
        v0 = c * VPAD
        nv = min(VPAD, VOCAB - v0)
        if nv <= 0:
            break
        # OUT is [(T*B), VPAD] fp16, row t*B + b
        o = np.asarray(res.results[c]["OUT"], dtype=np.float32)
        o = o.reshape(STEPS, BATCH, VPAD)
        full[:, v0:v0 + nv, :] = o[:, :, :nv].transpose(1, 2, 0)
    b_out = np.asarray(args["b_out"], np.float32)
    if np.any(b_out):
        full += b_out[None, :, None]
    return np.ascontiguousarray(full, dtype=np.float32)


# revision 9
# speedup vs baseline: 2.4901x; 1.3009x over previous
"""Trainium2 Bass kernel for the GRU caption model.

Computes: h0 = feat @ W_hp.T + b_hp; 200-step GRU with constant hidden-proj
gate pre-activations; logits = outs @ W_out.T (+ b_out on host) -> [B, V, T].

Sharding: hybrid 2-way batch x 4-way vocab across the 8 cores.  Core c
handles batch half c//4 (16 rows) and vocab quarter c%4 (7680 padded rows).
Each core runs its batch half's GRU; the projection uses the GRU state tiles
as the *stationary* matmul operand ([128 h, 128 (t,b)] chunks) and streams
W_out columns, so each 8-timestep "granule" yields a [128 (t,b), 7680 v]
fp16 tile that leaves in one large DMA.  PSUM->fp16 drains run on the
otherwise-idle GPSIMD engine so Act/DVE serve only the serial GRU chain.
Gate constants (W_hh @ h0 + biases) are accumulated into the gates PSUM
through a small identity matmul; the r-gate PSUM is a separate tile so the
chain's first tanh only waits on the r matmuls.
"""

import numpy as np
import ml_dtypes

import concourse.bass as bass
import concourse.mybir as mybir
import concourse.tile as tile
from concourse import bacc
from concourse.bass_utils import run_bass_kernel_spmd

F32 = mybir.dt.float32
BF16 = mybir.dt.bfloat16
FP16 = mybir.dt.float16
AF = mybir.ActivationFunctionType
ALU = mybir.AluOpType

VOCAB = 30522
HID = 512
FEAT = 2048
STEPS = 200
BATCH = 32
SOS = 101
NCORES = 8
P = 128
KO = HID // P            # 4 h-chunks
GM = 3 * HID // P        # 12 gate row-groups (r: 0-3, z: 4-7, n: 8-11)
KF = FEAT // P           # 16 feat chunks
BS = 16                  # per-core batch shard
NVQ = 4                  # vocab quarters
VPAD = 30720 // NVQ      # per-core padded vocab rows = 7680
GR = P // BS             # granule timesteps -> 128 (t,b) columns (8)
NGRAN = STEPS // GR      # 25
VC = 480                 # proj v-chunk columns (psum bank holds <=512 f32)
NVC = VPAD // VC         # 16 units per granule
UPS = NVC // GR          # proj units emitted per step (2)

LAST_RESULTS = None  # test harness introspection


def build():
    nc = bacc.Bacc("TRN2", target_bir_lowering=False, debug=False)

    featT = nc.dram_tensor("featT", [FEAT, BS], F32, kind="ExternalInput")
    WhpT = nc.dram_tensor("WhpT", [FEAT, HID], F32, kind="ExternalInput")
    WhhT = nc.dram_tensor("WhhT", [HID, 3 * HID], F32, kind="ExternalInput")
    WihT = nc.dram_tensor("WihT", [HID, 3 * HID], BF16, kind="ExternalInput")
    x0T = nc.dram_tensor("x0T", [HID, BS], BF16, kind="ExternalInput")
    WoutT = nc.dram_tensor("WoutT", [HID, VPAD], BF16, kind="ExternalInput")
    b_hp = nc.dram_tensor("b_hp", [HID], F32, kind="ExternalInput")
    bsum_rz = nc.dram_tensor("bsum_rz", [2 * HID], F32, kind="ExternalInput")
    bmix_n = nc.dram_tensor("bmix_n", [HID], F32, kind="ExternalInput")
    bhhn_half = nc.dram_tensor("bhhn_half", [HID], F32, kind="ExternalInput")
    I16 = nc.dram_tensor("I16", [BS, BS], BF16, kind="ExternalInput")
    I128 = nc.dram_tensor("I128", [P, P], F32, kind="ExternalInput")
    # row (t*BS + b) holds logits[b, :, t] for this core's vocab slice
    OUT = nc.dram_tensor("OUT", [STEPS * BS, VPAD], FP16, kind="ExternalOutput")

    with tile.TileContext(nc) as tc:
        with (
            tc.tile_pool(name="const", bufs=1) as const,
            tc.tile_pool(name="stage", bufs=2) as stagep,
            tc.tile_pool(name="step", bufs=3) as sp,
            tc.tile_pool(name="psr", bufs=2, space="PSUM") as psrp,
            tc.tile_pool(name="pszn", bufs=2, space="PSUM") as psznp,
            tc.tile_pool(name="psp", bufs=3, space="PSUM") as psp,
            tc.tile_pool(name="pst", bufs=1, space="PSUM") as pst,
        ):
            # ---- constants into SBUF ----
            wih = const.tile([P, KO, GM, P], BF16, tag="wih")
            nc.sync.dma_start(
                wih[:], WihT.rearrange("(k p) (m c) -> p k m c", p=P, c=P)
            )
            wout = const.tile([P, KO, VPAD], BF16, tag="wout")
            nc.sync.dma_start(wout[:], WoutT.rearrange("(k p) v -> p k v", p=P))
            whp_sb = const.tile([P, KF, HID], F32, tag="whp")
            nc.sync.dma_start(whp_sb[:], WhpT.rearrange("(k p) h -> p k h", p=P))
            whh_sb = const.tile([P, KO, 3 * HID], F32, tag="whh")
            nc.sync.dma_start(whh_sb[:], WhhT.rearrange("(k p) g -> p k g", p=P))
            featT_sb = const.tile([P, KF, BS], F32, tag="featsb")
            nc.sync.dma_start(featT_sb[:], featT.rearrange("(k p) b -> p k b", p=P))
            bhp_sb = const.tile([P, KO], F32, tag="bhp")
            nc.sync.dma_start(bhp_sb[:], b_hp.rearrange("(m p) -> p m", p=P))
            bsrz_sb = const.tile([P, 8], F32, tag="bsrz")
            nc.sync.dma_start(bsrz_sb[:], bsum_rz.rearrange("(m p) -> p m", p=P))
            bmixn_sb = const.tile([P, KO], F32, tag="bmixn")
            nc.sync.dma_start(bmixn_sb[:], bmix_n.rearrange("(m p) -> p m", p=P))
            bhhnh_sb = const.tile([P, KO], F32, tag="bhhnh")
            nc.sync.dma_start(bhhnh_sb[:], bhhn_half.rearrange("(m p) -> p m", p=P))
            i16_sb = const.tile([BS, BS], BF16, tag="i16")
            nc.sync.dma_start(i16_sb[:], I16[:, :])
            i128_sb = const.tile([P, P], F32, tag="i128")
            nc.sync.dma_start(i128_sb[:], I128[:, :])
            x0_sb = const.tile([P, KO, BS], BF16, tag="x0")
            nc.sync.dma_start(x0_sb[:], x0T.rearrange("(k p) b -> p k b", p=P))

            # resT[p, k, t, b] = h_{t+1}[k*128+p, b]; (t, b) last so an
            # 8-step granule slice is a contiguous 128-column stationary
            # operand for the projection matmuls.
            resT = const.tile([P, KO, STEPS, BS], BF16, tag="resT")

            # ---- h0 = feat @ W_hp.T + b_hp (fp32, exact) ----
            ps_h = psznp.tile([P, 8, BS], F32, tag="gzn")
            for ko in range(KO):
                for kf in range(KF):
                    nc.tensor.matmul(
                        ps_h[:, ko, :],
                        whp_sb[:, kf, ko * P:(ko + 1) * P],
                        featT_sb[:, kf, :],
                        start=(kf == 0), stop=(kf == KF - 1),
                    )
            h0T = const.tile([P, KO, BS], F32, tag="h0T")
            for ko in range(KO):
                nc.scalar.activation(
                    h0T[:, ko, :], ps_h[:, ko, :], AF.Identity,
                    bias=bhp_sb[:, ko, None], scale=1.0,
                )
            h0h = const.tile([P, KO, BS], BF16, tag="h0h")
            nc.scalar.mul(h0h[:], h0T[:], 0.5)

            # ---- gate constants G0 (fp32 psum -> transposed bf16) ----
            # rz rows: G0 = W_hh@h0 + b_hh + b_ih
            # n rows:  G0 = 0.5*(W_hh@h0 + b_hh) + b_ih   (E_n form)
            ps_rz = psznp.tile([P, 8, BS], F32, tag="gzn")
            for m in range(8):
                for k in range(KO):
                    nc.tensor.matmul(
                        ps_rz[:, m, :],
                        whh_sb[:, k, m * P:(m + 1) * P],
                        h0T[:, k, :],
                        start=(k == 0), stop=(k == KO - 1),
                    )
            ps_n = psrp.tile([P, 4, BS], F32, tag="gr")
            for m in range(4):
                for k in range(KO):
                    nc.tensor.matmul(
                        ps_n[:, m, :],
                        whh_sb[:, k, (m + 8) * P:(m + 9) * P],
                        h0T[:, k, :],
                        start=(k == 0), stop=(k == KO - 1),
                    )
            G0 = const.tile([P, GM, BS], F32, tag="G0")
            nc.vector.tensor_add(
                G0[:, 0:8, :], ps_rz[:],
                bsrz_sb[:, :, None].to_broadcast((P, 8, BS)),
            )
            nc.vector.scalar_tensor_tensor(
                G0[:, 8:12, :], ps_n[:], 0.5,
                bmixn_sb[:, :, None].to_broadcast((P, KO, BS)),
                ALU.mult, ALU.add,
            )
            # hn2 = 0.5*(W_hh@h0 + b_hh)_n
            hn2 = const.tile([P, KO, BS], BF16, tag="hn2")
            nc.vector.scalar_tensor_tensor(
                hn2[:], ps_n[:], 0.5,
                bhhnh_sb[:, :, None].to_broadcast((P, KO, BS)),
                ALU.mult, ALU.add,
            )
            # G0T[b, m*128+p] = G0[p, m, b], bf16 for the per-step const matmul
            G0T = const.tile([BS, GM * P], BF16, tag="G0T")
            for m in range(GM):
                tp = pst.tile([BS, P], F32, tag="tp")
                nc.tensor.transpose(tp[:], G0[:, m, :], i128_sb[:])
                nc.scalar.copy(G0T[:, m * P:(m + 1) * P], tp[:])

            # ---- per-granule projection state ----
            stage_tiles = {}

            def emit_proj_mm(g, u):
                if u == 0:
                    stage_tiles[g] = stagep.tile(
                        [P, VPAD], FP16, tag="stage", name=f"stage{g}"
                    )
                pp = psp.tile([P, VC], F32, tag="pp", name=f"pp{g}_{u}")
                for k in range(KO):
                    nc.tensor.matmul(
                        pp[:],
                        resT[:, k, g * GR:(g + 1) * GR, :],
                        wout[:, k, u * VC:(u + 1) * VC],
                        start=(k == 0), stop=(k == KO - 1),
                    )
                return pp

            def emit_proj_tail(g, u, pp):
                # Pool engine is otherwise idle; keeping the PSUM->fp16 drains
                # off Act/DVE protects the serial GRU chain from head-of-line
                # blocking in those engine queues.
                st = stage_tiles[g]
                nc.gpsimd.tensor_scalar_add(st[:, u * VC:(u + 1) * VC], pp[:], 0.0)
                if u == NVC - 1:
                    nc.sync.dma_start(OUT[g * P:(g + 1) * P, :], st[:])
                    del stage_tiles[g]

            # ---- GRU steps ----
            for t in range(STEPS):
                psr = psrp.tile([P, 4, BS], F32, tag="gr")
                pszn = psznp.tile([P, 8, BS], F32, tag="gzn")
                for m in range(GM):
                    dst = psr[:, m, :] if m < 4 else pszn[:, m - 4, :]
                    nc.tensor.matmul(
                        dst, G0T[:, m * P:(m + 1) * P], i16_sb[:],
                        start=True, stop=False,
                    )
                    for k in range(KO):
                        rhs = x0_sb[:, k, :] if t == 0 else resT[:, k, t - 1, :]
                        nc.tensor.matmul(
                            dst, wih[:, k, m, :], rhs,
                            start=False, stop=(k == KO - 1),
                        )
                # projection units (granule g = t//GR - 1) interleave here to
                # fill the PE stream while the elementwise chain runs
                pps = []
                g = t // GR - 1
                if g >= 0:
                    for u in range(UPS * (t % GR), UPS * (t % GR) + UPS):
                        pps.append((g, u, emit_proj_mm(g, u)))

                tr = sp.tile([P, KO, BS], BF16, tag="tr")
                nc.scalar.activation(tr[:], psr[:], AF.Tanh, scale=0.5)
                tz = sp.tile([P, KO, BS], BF16, tag="tz")
                nc.scalar.activation(tz[:], pszn[:, 0:4, :], AF.Tanh, scale=0.5)
                a = sp.tile([P, KO, BS], BF16, tag="a")
                nc.vector.tensor_mul(a[:], tr[:], hn2[:])
                sn = sp.tile([P, KO, BS], BF16, tag="sn")
                nc.vector.tensor_add(sn[:], pszn[:, 4:8, :], a[:])
                n_ = sp.tile([P, KO, BS], BF16, tag="n")
                nc.scalar.activation(n_[:], sn[:], AF.Tanh, scale=1.0)
                # d = 0.5 - 0.5*tz ; c1 = h0h*(1+tz) = h0h + h0h*tz
                d = sp.tile([P, KO, BS], BF16, tag="d")
                nc.vector.tensor_scalar(d[:], tz[:], -0.5, 0.5, ALU.mult, ALU.add)
                u_ = sp.tile([P, KO, BS], BF16, tag="u")
                nc.vector.tensor_mul(u_[:], tz[:], h0h[:])
                c1 = sp.tile([P, KO, BS], BF16, tag="c1")
                nc.vector.tensor_add(c1[:], u_[:], h0h[:])
                e = sp.tile([P, KO, BS], BF16, tag="e")
                nc.vector.tensor_mul(e[:], n_[:], d[:])
                # h' = e + c1, written straight into the res history
                nc.vector.tensor_add(resT[:, :, t, :], e[:], c1[:])

                for g, u, pp in pps:
                    emit_proj_tail(g, u, pp)

            # ---- drain the last granule's projection ----
            g = NGRAN - 1
            for u in range(NVC):
                pp = emit_proj_mm(g, u)
                emit_proj_tail(g, u, pp)

    nc.compile()
    return nc


def _shard_inputs(feat, W_hp, b_hp, W_ih, W_hh, b_ih, b_hh, embed, W_out, b_out):
    bf = ml_dtypes.bfloat16
    feat = np.asarray(feat)
    WhpT = np.ascontiguousarray(np.asarray(W_hp).T, dtype=np.float32)
    WhhT = np.ascontiguousarray(np.asarray(W_hh).T, dtype=np.float32)
    WihT = np.ascontiguousarray(np.asarray(W_ih).T).astype(bf)
    x0T = np.ascontiguousarray(
        np.repeat(np.asarray(embed)[SOS][:, None], BS, axis=1)
    ).astype(bf)
    b_ih = np.asarray(b_ih, np.float32)
    b_hh = np.asarray(b_hh, np.float32)
    bsum_rz = (b_hh + b_ih)[:2 * HID].copy()
    bmix_n = (0.5 * b_hh + b_ih)[2 * HID:].copy()
    bhhn_half = (0.5 * b_hh)[2 * HID:].copy()
    Wo = np.zeros((NVQ * VPAD, HID), np.float32)
    Wo[:VOCAB] = np.asarray(W_out)
    common = dict(
        WhpT=WhpT, WhhT=WhhT, WihT=WihT, x0T=x0T,
        b_hp=np.asarray(b_hp, np.float32),
        bsum_rz=bsum_rz, bmix_n=bmix_n, bhhn_half=bhhn_half,
        I16=np.eye(BS, dtype=np.float32).astype(bf),
        I128=np.eye(P, dtype=np.float32),
    )
    featT_halves = [
        np.ascontiguousarray(feat[hb * BS:(hb + 1) * BS].T, dtype=np.float32)
        for hb in range(2)
    ]
    woutT_quarters = [
        np.ascontiguousarray(Wo[vq * VPAD:(vq + 1) * VPAD].T).astype(bf)
        for vq in range(NVQ)
    ]
    in_maps = []
    for c in range(NCORES):
        hb, vq = divmod(c, NVQ)
        m = dict(common)
        m["featT"] = featT_halves[hb]
        m["WoutT"] = woutT_quarters[vq]
        in_maps.append(m)
    return in_maps


def kernel(**inputs):
    global LAST_RESULTS
    args = {k: np.asarray(v) for k, v in inputs.items()}
    in_maps = _shard_inputs(
        args["feat"], args["W_hp"], args["b_hp"], args["W_ih"], args["W_hh"],
        args["b_ih"], args["b_hh"], args["embed"], args["W_out"], args["b_out"],
    )
    nc = build()
    res = run_bass_kernel_spmd(nc, in_maps, core_ids=list(range(NCORES)))
    LAST_RESULTS = res
    full = np.empty((BATCH, VOCAB, STEPS), np.float32)
    for c in range(NCORES):
        hb, vq = divmod(c, NVQ)
        v0 = vq * VPAD
        nv = min(VPAD, VOCAB - v0)
        if nv <= 0:
            continue
        # OUT is [(T*BS), VPAD] fp16, row t*BS + b
        o = np.asarray(res.results[c]["OUT"], dtype=np.float32)
        o = o.reshape(STEPS, BS, VPAD)
        full[hb * BS:(hb + 1) * BS, v0:v0 + nv, :] = (
            o[:, :, :nv].transpose(1, 2, 0)
        )
    b_out = np.asarray(args["b_out"], np.float32)
    if np.any(b_out):
        full += b_out[None, :, None]
    return np.ascontiguousarray(full, dtype=np.float32)


# revision 11
# speedup vs baseline: 2.6053x; 1.0463x over previous
"""Trainium2 Bass kernel for the GRU caption model.

Computes: h0 = feat @ W_hp.T + b_hp; 200-step GRU with constant hidden-proj
gate pre-activations; logits = outs @ W_out.T (+ b_out on host) -> [B, V, T].

Sharding: hybrid 2-way batch x 4-way vocab across the 8 cores.  Core c
handles batch half c//4 (16 rows) and vocab quarter c%4 (7680 padded rows).
Each core runs its batch half's GRU; the projection uses the GRU state tiles
as the *stationary* matmul operand ([128 h, 128 (t,b)] chunks) and streams
W_out columns, so each 8-timestep "granule" yields a [128 (t,b), 7680 v]
fp16 tile that leaves in one large DMA.  PSUM->fp16 drains run on the
otherwise-idle GPSIMD engine so Act/DVE serve only the serial GRU chain.
Gate constants (W_hh @ h0 + biases) are accumulated into the gates PSUM
through a small identity matmul; the r-gate PSUM is a separate tile so the
chain's first tanh only waits on the r matmuls.
"""

import numpy as np
import ml_dtypes

import concourse.bass as bass
import concourse.mybir as mybir
import concourse.tile as tile
from concourse import bacc
from concourse.bass_utils import run_bass_kernel_spmd

F32 = mybir.dt.float32
BF16 = mybir.dt.bfloat16
FP16 = mybir.dt.float16
AF = mybir.ActivationFunctionType
ALU = mybir.AluOpType

VOCAB = 30522
HID = 512
FEAT = 2048
STEPS = 200
BATCH = 32
SOS = 101
NCORES = 8
P = 128
KO = HID // P            # 4 h-chunks
GM = 3 * HID // P        # 12 gate row-groups (r: 0-3, z: 4-7, n: 8-11)
KF = FEAT // P           # 16 feat chunks
BS = 16                  # per-core batch shard
NVQ = 4                  # vocab quarters
VPAD = 30720 // NVQ      # per-core padded vocab rows = 7680
GR = P // BS             # granule timesteps -> 128 (t,b) columns (8)
NGRAN = STEPS // GR      # 25
VC = 480                 # proj v-chunk columns (psum bank holds <=512 f32)
NVC = VPAD // VC         # 16 units per granule
UPS = NVC // GR          # proj units emitted per step (2)

LAST_RESULTS = None  # test harness introspection


def build():
    nc = bacc.Bacc("TRN2", target_bir_lowering=False, debug=False)

    featT = nc.dram_tensor("featT", [FEAT, BS], F32, kind="ExternalInput")
    WhpT = nc.dram_tensor("WhpT", [FEAT, HID], F32, kind="ExternalInput")
    WhhT = nc.dram_tensor("WhhT", [HID, 3 * HID], F32, kind="ExternalInput")
    WihT = nc.dram_tensor("WihT", [HID, 3 * HID], BF16, kind="ExternalInput")
    x0T = nc.dram_tensor("x0T", [HID, BS], BF16, kind="ExternalInput")
    WoutT = nc.dram_tensor("WoutT", [HID, VPAD], BF16, kind="ExternalInput")
    b_hp = nc.dram_tensor("b_hp", [HID], F32, kind="ExternalInput")
    bsum_rz = nc.dram_tensor("bsum_rz", [2 * HID], F32, kind="ExternalInput")
    bmix_n = nc.dram_tensor("bmix_n", [HID], F32, kind="ExternalInput")
    bhhn_half = nc.dram_tensor("bhhn_half", [HID], F32, kind="ExternalInput")
    I16 = nc.dram_tensor("I16", [BS, BS], BF16, kind="ExternalInput")
    I128 = nc.dram_tensor("I128", [P, P], F32, kind="ExternalInput")
    # row (t*BS + b) holds logits[b, :, t] for this core's vocab slice
    OUT = nc.dram_tensor("OUT", [STEPS * BS, VPAD], FP16, kind="ExternalOutput")

    with tile.TileContext(nc) as tc:
        with (
            tc.tile_pool(name="const", bufs=1) as const,
            tc.tile_pool(name="stage", bufs=2) as stagep,
            tc.tile_pool(name="step", bufs=3) as sp,
            tc.tile_pool(name="psr", bufs=2, space="PSUM") as psrp,
            tc.tile_pool(name="pszn", bufs=2, space="PSUM") as psznp,
            tc.tile_pool(name="psp", bufs=3, space="PSUM") as psp,
            tc.tile_pool(name="pst", bufs=1, space="PSUM") as pst,
        ):
            # ---- constants into SBUF ----
            # DMA_ENGINES serialize transfers, so order by when each tensor
            # is first needed: feat/whp (h0) -> whh (G0) -> wih/x0 (step 0);
            # the big wout load is only needed once projection starts (t>=8).
            featT_sb = const.tile([P, KF, BS], F32, tag="featsb")
            nc.sync.dma_start(featT_sb[:], featT.rearrange("(k p) b -> p k b", p=P))
            whp_sb = const.tile([P, KF, HID], F32, tag="whp")
            nc.sync.dma_start(whp_sb[:], WhpT.rearrange("(k p) h -> p k h", p=P))
            whh_sb = const.tile([P, KO, 3 * HID], F32, tag="whh")
            nc.sync.dma_start(whh_sb[:], WhhT.rearrange("(k p) g -> p k g", p=P))
            wih = const.tile([P, KO, GM, P], BF16, tag="wih")
            nc.sync.dma_start(
                wih[:], WihT.rearrange("(k p) (m c) -> p k m c", p=P, c=P)
            )
            bhp_sb = const.tile([P, KO], F32, tag="bhp")
            nc.sync.dma_start(bhp_sb[:], b_hp.rearrange("(m p) -> p m", p=P))
            bsrz_sb = const.tile([P, 8], F32, tag="bsrz")
            nc.sync.dma_start(bsrz_sb[:], bsum_rz.rearrange("(m p) -> p m", p=P))
            bmixn_sb = const.tile([P, KO], F32, tag="bmixn")
            nc.sync.dma_start(bmixn_sb[:], bmix_n.rearrange("(m p) -> p m", p=P))
            bhhnh_sb = const.tile([P, KO], F32, tag="bhhnh")
            nc.sync.dma_start(bhhnh_sb[:], bhhn_half.rearrange("(m p) -> p m", p=P))
            i16_sb = const.tile([BS, BS], BF16, tag="i16")
            nc.sync.dma_start(i16_sb[:], I16[:, :])
            i128_sb = const.tile([P, P], F32, tag="i128")
            nc.sync.dma_start(i128_sb[:], I128[:, :])
            x0_sb = const.tile([P, KO, BS], BF16, tag="x0")
            nc.sync.dma_start(x0_sb[:], x0T.rearrange("(k p) b -> p k b", p=P))
            wout = const.tile([P, KO, VPAD], BF16, tag="wout")
            nc.sync.dma_start(wout[:], WoutT.rearrange("(k p) v -> p k v", p=P))

            # resT[p, k, t, b] = h_{t+1}[k*128+p, b]; (t, b) last so an
            # 8-step granule slice is a contiguous 128-column stationary
            # operand for the projection matmuls.
            resT = const.tile([P, KO, STEPS, BS], BF16, tag="resT")

            # ---- h0 = feat @ W_hp.T + b_hp (fp32, exact) ----
            ps_h = psznp.tile([P, 8, BS], F32, tag="gzn")
            for ko in range(KO):
                for kf in range(KF):
                    nc.tensor.matmul(
                        ps_h[:, ko, :],
                        whp_sb[:, kf, ko * P:(ko + 1) * P],
                        featT_sb[:, kf, :],
                        start=(kf == 0), stop=(kf == KF - 1),
                    )
            h0T = const.tile([P, KO, BS], F32, tag="h0T")
            for ko in range(KO):
                nc.scalar.activation(
                    h0T[:, ko, :], ps_h[:, ko, :], AF.Identity,
                    bias=bhp_sb[:, ko, None], scale=1.0,
                )
            h0h = const.tile([P, KO, BS], BF16, tag="h0h")
            nc.scalar.mul(h0h[:], h0T[:], 0.5)

            # ---- gate constants G0 (fp32 psum -> transposed bf16) ----
            # rz rows: G0 = W_hh@h0 + b_hh + b_ih
            # n rows:  G0 = 0.5*(W_hh@h0 + b_hh) + b_ih   (E_n form)
            ps_rz = psznp.tile([P, 8, BS], F32, tag="gzn")
            for m in range(8):
                for k in range(KO):
                    nc.tensor.matmul(
                        ps_rz[:, m, :],
                        whh_sb[:, k, m * P:(m + 1) * P],
                        h0T[:, k, :],
                        start=(k == 0), stop=(k == KO - 1),
                    )
            ps_n = psrp.tile([P, 4, BS], F32, tag="gr")
            for m in range(4):
                for k in range(KO):
                    nc.tensor.matmul(
                        ps_n[:, m, :],
                        whh_sb[:, k, (m + 8) * P:(m + 9) * P],
                        h0T[:, k, :],
                        start=(k == 0), stop=(k == KO - 1),
                    )
            G0 = const.tile([P, GM, BS], F32, tag="G0")
            nc.vector.tensor_add(
                G0[:, 0:8, :], ps_rz[:],
                bsrz_sb[:, :, None].to_broadcast((P, 8, BS)),
            )
            nc.vector.scalar_tensor_tensor(
                G0[:, 8:12, :], ps_n[:], 0.5,
                bmixn_sb[:, :, None].to_broadcast((P, KO, BS)),
                ALU.mult, ALU.add,
            )
            # hn2 = 0.5*(W_hh@h0 + b_hh)_n
            hn2 = const.tile([P, KO, BS], BF16, tag="hn2")
            nc.vector.scalar_tensor_tensor(
                hn2[:], ps_n[:], 0.5,
                bhhnh_sb[:, :, None].to_broadcast((P, KO, BS)),
                ALU.mult, ALU.add,
            )
            # G0T[b, m*128+p] = G0[p, m, b], bf16 for the per-step const matmul
            G0T = const.tile([BS, GM * P], BF16, tag="G0T")
            for m in range(GM):
                tp = pst.tile([BS, P], F32, tag="tp")
                nc.tensor.transpose(tp[:], G0[:, m, :], i128_sb[:])
                nc.scalar.copy(G0T[:, m * P:(m + 1) * P], tp[:])

            # ---- per-granule projection state ----
            stage_tiles = {}

            def emit_proj_mm(g, u):
                if u == 0:
                    stage_tiles[g] = stagep.tile(
                        [P, VPAD], FP16, tag="stage", name=f"stage{g}"
                    )
                pp = psp.tile([P, VC], F32, tag="pp", name=f"pp{g}_{u}")
                for k in range(KO):
                    nc.tensor.matmul(
                        pp[:],
                        resT[:, k, g * GR:(g + 1) * GR, :],
                        wout[:, k, u * VC:(u + 1) * VC],
                        start=(k == 0), stop=(k == KO - 1),
                    )
                return pp

            def emit_proj_tail(g, u, pp):
                # Pool engine is otherwise idle; keeping the PSUM->fp16 drains
                # off Act/DVE protects the serial GRU chain from head-of-line
                # blocking in those engine queues.
                st = stage_tiles[g]
                nc.gpsimd.tensor_scalar_add(st[:, u * VC:(u + 1) * VC], pp[:], 0.0)
                if u == NVC - 1:
                    nc.sync.dma_start(OUT[g * P:(g + 1) * P, :], st[:])
                    del stage_tiles[g]

            # ---- GRU steps ----
            for t in range(STEPS):
                psr = psrp.tile([P, 4, BS], F32, tag="gr")
                pszn = psznp.tile([P, 8, BS], F32, tag="gzn")
                for m in range(GM):
                    dst = psr[:, m, :] if m < 4 else pszn[:, m - 4, :]
                    nc.tensor.matmul(
                        dst, G0T[:, m * P:(m + 1) * P], i16_sb[:],
                        start=True, stop=False,
                    )
                    for k in range(KO):
                        rhs = x0_sb[:, k, :] if t == 0 else resT[:, k, t - 1, :]
                        nc.tensor.matmul(
                            dst, wih[:, k, m, :], rhs,
                            start=False, stop=(k == KO - 1),
                        )
                # projection units (granule g = t//GR - 1) interleave here to
                # fill the PE stream while the elementwise chain runs
                pps = []
                g = t // GR - 1
                if g >= 0:
                    for u in range(UPS * (t % GR), UPS * (t % GR) + UPS):
                        pps.append((g, u, emit_proj_mm(g, u)))

                tr = sp.tile([P, KO, BS], BF16, tag="tr")
                nc.scalar.activation(tr[:], psr[:], AF.Tanh, scale=0.5)
                tz = sp.tile([P, KO, BS], BF16, tag="tz")
                nc.scalar.activation(tz[:], pszn[:, 0:4, :], AF.Tanh, scale=0.5)
                a = sp.tile([P, KO, BS], BF16, tag="a")
                nc.vector.tensor_mul(a[:], tr[:], hn2[:])
                sn = sp.tile([P, KO, BS], BF16, tag="sn")
                nc.vector.tensor_add(sn[:], pszn[:, 4:8, :], a[:])
                n_ = sp.tile([P, KO, BS], BF16, tag="n")
                nc.scalar.activation(n_[:], sn[:], AF.Tanh, scale=1.0)
                # d = 0.5 - 0.5*tz ; c1 = h0h*(1+tz) = h0h + h0h*tz
                d = sp.tile([P, KO, BS], BF16, tag="d")
                nc.vector.tensor_scalar(d[:], tz[:], -0.5, 0.5, ALU.mult, ALU.add)
                u_ = sp.tile([P, KO, BS], BF16, tag="u")
                nc.vector.tensor_mul(u_[:], tz[:], h0h[:])
                c1 = sp.tile([P, KO, BS], BF16, tag="c1")
                nc.vector.tensor_add(c1[:], u_[:], h0h[:])
                e = sp.tile([P, KO, BS], BF16, tag="e")
                nc.vector.tensor_mul(e[:], n_[:], d[:])
                # h' = e + c1, written straight into the res history
                nc.vector.tensor_add(resT[:, :, t, :], e[:], c1[:])

                for g, u, pp in pps:
                    emit_proj_tail(g, u, pp)

            # ---- drain the last granule's projection ----
            g = NGRAN - 1
            for u in range(NVC):
                pp = emit_proj_mm(g, u)
                emit_proj_tail(g, u, pp)

    nc.compile()
    return nc


def _shard_inputs(feat, W_hp, b_hp, W_ih, W_hh, b_ih, b_hh, embed, W_out, b_out):
    bf = ml_dtypes.bfloat16
    feat = np.asarray(feat)
    WhpT = np.ascontiguousarray(np.asarray(W_hp).T, dtype=np.float32)
    WhhT = np.ascontiguousarray(np.asarray(W_hh).T, dtype=np.float32)
    WihT = np.ascontiguousarray(np.asarray(W_ih).T).astype(bf)
    x0T = np.ascontiguousarray(
        np.repeat(np.asarray(embed)[SOS][:, None], BS, axis=1)
    ).astype(bf)
    b_ih = np.asarray(b_ih, np.float32)
    b_hh = np.asarray(b_hh, np.float32)
    bsum_rz = (b_hh + b_ih)[:2 * HID].copy()
    bmix_n = (0.5 * b_hh + b_ih)[2 * HID:].copy()
    bhhn_half = (0.5 * b_hh)[2 * HID:].copy()
    Wo = np.zeros((NVQ * VPAD, HID), np.float32)
    Wo[:VOCAB] = np.asarray(W_out)
    common = dict(
        WhpT=WhpT, WhhT=WhhT, WihT=WihT, x0T=x0T,
        b_hp=np.asarray(b_hp, np.float32),
        bsum_rz=bsum_rz, bmix_n=bmix_n, bhhn_half=bhhn_half,
        I16=np.eye(BS, dtype=np.float32).astype(bf),
        I128=np.eye(P, dtype=np.float32),
    )
    featT_halves = [
        np.ascontiguousarray(feat[hb * BS:(hb + 1) * BS].T, dtype=np.float32)
        for hb in range(2)
    ]
    woutT_quarters = [
        np.ascontiguousarray(Wo[vq * VPAD:(vq + 1) * VPAD].T).astype(bf)
        for vq in range(NVQ)
    ]
    in_maps = []
    for c in range(NCORES):
        hb, vq = divmod(c, NVQ)
        m = dict(common)
        m["featT"] = featT_halves[hb]
        m["WoutT"] = woutT_quarters[vq]
        in_maps.append(m)
    return in_maps


def kernel(**inputs):
    global LAST_RESULTS
    args = {k: np.asarray(v) for k, v in inputs.items()}
    in_maps = _shard_inputs(
        args["feat"], args["W_hp"], args["b_hp"], args["W_ih"], args["W_hh"],
        args["b_ih"], args["b_hh"], args["embed"], args["W_out"], args["b_out"],
    )
    nc = build()
    res = run_bass_kernel_spmd(nc, in_maps, core_ids=list(range(NCORES)))
    LAST_RESULTS = res
    full = np.empty((BATCH, VOCAB, STEPS), np.float32)
    for c in range(NCORES):
        hb, vq = divmod(c, NVQ)
        v0 = vq * VPAD
        nv = min(VPAD, VOCAB - v0)
        if nv <= 0:
            continue
        # OUT is [(T*BS), VPAD] fp16, row t*BS + b
        o = np.asarray(res.results[c]["OUT"], dtype=np.float32)
        o = o.reshape(STEPS, BS, VPAD)
        full[hb * BS:(hb + 1) * BS, v0:v0 + nv, :] = (
            o[:, :, :nv].transpose(1, 2, 0)
        )
    b_out = np.asarray(args["b_out"], np.float32)
    if np.any(b_out):
        full += b_out[None, :, None]
    return np.ascontiguousarray(full, dtype=np.float32)


# revision 17
# speedup vs baseline: 2.6174x; 1.0046x over previous
"""Trainium2 Bass kernel for the GRU caption model.

Computes: h0 = feat @ W_hp.T + b_hp; 200-step GRU with constant hidden-proj
gate pre-activations; logits = outs @ W_out.T (+ b_out on host) -> [B, V, T].

Sharding: hybrid 2-way batch x 4-way vocab across the 8 cores.  Core c
handles batch half c//4 (16 rows) and vocab quarter c%4 (7680 padded rows).
Each core runs its batch half's GRU; the projection uses the GRU state tiles
as the *stationary* matmul operand ([128 h, 128 (t,b)] chunks) and streams
W_out columns, so each 8-timestep "granule" yields a [128 (t,b), 7680 v]
fp16 tile that leaves in one large DMA.  PSUM->fp16 drains run on the
otherwise-idle GPSIMD engine so Act/DVE serve only the serial GRU chain.
Gate constants (W_hh @ h0 + biases) are accumulated into the gates PSUM
through a small identity matmul; the r-gate PSUM is a separate tile so the
chain's first tanh only waits on the r matmuls.
"""

import numpy as np
import ml_dtypes

import concourse.bass as bass
import concourse.mybir as mybir
import concourse.tile as tile
from concourse import bacc
from concourse.bass_utils import run_bass_kernel_spmd

F32 = mybir.dt.float32
BF16 = mybir.dt.bfloat16
FP16 = mybir.dt.float16
AF = mybir.ActivationFunctionType
ALU = mybir.AluOpType

VOCAB = 30522
HID = 512
FEAT = 2048
STEPS = 200
BATCH = 32
SOS = 101
NCORES = 8
P = 128
KO = HID // P            # 4 h-chunks
GM = 3 * HID // P        # 12 gate row-groups (r: 0-3, z: 4-7, n: 8-11)
KF = FEAT // P           # 16 feat chunks
BS = 16                  # per-core batch shard
NVQ = 4                  # vocab quarters
VPAD = 30720 // NVQ      # per-core padded vocab rows = 7680
GR = P // BS             # granule timesteps -> 128 (t,b) columns (8)
NGRAN = STEPS // GR      # 25
VC = 480                 # proj v-chunk columns (psum bank holds <=512 f32)
NVC = VPAD // VC         # 16 units per granule
UPS = NVC // GR          # proj units emitted per step (2)

LAST_RESULTS = None  # test harness introspection


def build():
    nc = bacc.Bacc("TRN2", target_bir_lowering=False, debug=False)

    featT = nc.dram_tensor("featT", [FEAT, BS], F32, kind="ExternalInput")
    WhpT = nc.dram_tensor("WhpT", [FEAT, HID], F32, kind="ExternalInput")
    WhhT = nc.dram_tensor("WhhT", [HID, 3 * HID], BF16, kind="ExternalInput")
    WihT = nc.dram_tensor("WihT", [HID, 3 * HID], BF16, kind="ExternalInput")
    x0T = nc.dram_tensor("x0T", [HID, BS], BF16, kind="ExternalInput")
    WoutT = nc.dram_tensor("WoutT", [HID, VPAD], BF16, kind="ExternalInput")
    b_hp = nc.dram_tensor("b_hp", [HID], F32, kind="ExternalInput")
    # gate-constant bias row: [b_hh+b_ih (rz) | 0.5*b_hh+b_ih (n)]
    gbias = nc.dram_tensor("gbias", [3 * HID], BF16, kind="ExternalInput")
    bhhn_half = nc.dram_tensor("bhhn_half", [HID], F32, kind="ExternalInput")
    I16 = nc.dram_tensor("I16", [BS, BS], BF16, kind="ExternalInput")
    # row (t*BS + b) holds logits[b, :, t] for this core's vocab slice
    OUT = nc.dram_tensor("OUT", [STEPS * BS, VPAD], FP16, kind="ExternalOutput")

    with tile.TileContext(nc) as tc:
        with (
            tc.tile_pool(name="const", bufs=1) as const,
            tc.tile_pool(name="stage", bufs=2) as stagep,
            tc.tile_pool(name="step", bufs=3) as sp,
            tc.tile_pool(name="psr", bufs=2, space="PSUM") as psrp,
            tc.tile_pool(name="pszn", bufs=2, space="PSUM") as psznp,
            tc.tile_pool(name="psp", bufs=3, space="PSUM") as psp,
            tc.tile_pool(name="pst", bufs=1, space="PSUM") as pst,
        ):
            # ---- constants into SBUF ----
            # DMA_ENGINES serialize transfers, so order by when each tensor
            # is first needed: feat/whp (h0) -> whh (G0) -> wih/x0 (step 0);
            # the big wout load is only needed once projection starts (t>=8).
            featT_sb = const.tile([P, KF, BS], F32, tag="featsb")
            nc.sync.dma_start(featT_sb[:], featT.rearrange("(k p) b -> p k b", p=P))
            whp_sb = const.tile([P, KF, HID], F32, tag="whp")
            nc.sync.dma_start(whp_sb[:], WhpT.rearrange("(k p) h -> p k h", p=P))
            whh_sb = const.tile([P, KO, 3 * HID], BF16, tag="whh")
            nc.sync.dma_start(whh_sb[:], WhhT.rearrange("(k p) g -> p k g", p=P))
            wih = const.tile([P, KO, GM, P], BF16, tag="wih")
            nc.sync.dma_start(
                wih[:], WihT.rearrange("(k p) (m c) -> p k m c", p=P, c=P)
            )
            bhp_sb = const.tile([P, KO], F32, tag="bhp")
            nc.sync.dma_start(bhp_sb[:], b_hp.rearrange("(m p) -> p m", p=P))
            gb_sb = const.tile([1, 3 * HID], BF16, tag="gb")
            nc.sync.dma_start(gb_sb[:], gbias[None, :])
            bhhnh_sb = const.tile([P, KO], F32, tag="bhhnh")
            nc.sync.dma_start(bhhnh_sb[:], bhhn_half.rearrange("(m p) -> p m", p=P))
            i16_sb = const.tile([BS, BS], BF16, tag="i16")
            nc.sync.dma_start(i16_sb[:], I16[:, :])
            ones16 = const.tile([1, BS], BF16, tag="ones16")
            nc.vector.memset(ones16[:], 1.0)
            x0_sb = const.tile([P, KO, BS], BF16, tag="x0")
            nc.sync.dma_start(x0_sb[:], x0T.rearrange("(k p) b -> p k b", p=P))
            wout = const.tile([P, KO, VPAD], BF16, tag="wout")
            nc.sync.dma_start(wout[:], WoutT.rearrange("(k p) v -> p k v", p=P))

            # resT[p, k, t, b] = h_{t+1}[k*128+p, b]; (t, b) last so an
            # 8-step granule slice is a contiguous 128-column stationary
            # operand for the projection matmuls.
            resT = const.tile([P, KO, STEPS, BS], BF16, tag="resT")

            # ---- h0 = feat @ W_hp.T + b_hp (fp32, exact) ----
            ps_h = psznp.tile([P, 8, BS], F32, tag="gzn")
            for ko in range(KO):
                for kf in range(KF):
                    nc.tensor.matmul(
                        ps_h[:, ko, :],
                        whp_sb[:, kf, ko * P:(ko + 1) * P],
                        featT_sb[:, kf, :],
                        start=(kf == 0), stop=(kf == KF - 1),
                    )
            h0T = const.tile([P, KO, BS], F32, tag="h0T")
            for ko in range(KO):
                nc.scalar.activation(
                    h0T[:, ko, :], ps_h[:, ko, :], AF.Identity,
                    bias=bhp_sb[:, ko, None], scale=1.0,
                )
            h0h = const.tile([P, KO, BS], BF16, tag="h0h")
            nc.scalar.mul(h0h[:], h0T[:], 0.5)
            h0b = const.tile([P, KO, BS], BF16, tag="h0b")
            nc.scalar.copy(h0b[:], h0T[:])

            # ---- gate constants, computed directly transposed ----
            # G0T[b, g] = (h0 @ W_hh.T)[b, g] + bias[g], with the n third
            # using 0.5*h0 and the E_n bias mix (folded in via gbias and an
            # extra ones-row matmul so bias rides the PSUM accumulation).
            G0T = const.tile([BS, GM * P], BF16, tag="G0T")
            for gc in range(3):
                cs = slice(gc * 512, (gc + 1) * 512)
                lhs = h0b if gc < 2 else h0h
                ps_t = pst.tile([BS, 512], F32, tag="tp")
                for k in range(KO):
                    nc.tensor.matmul(
                        ps_t[:], lhs[:, k, :], whh_sb[:, k, cs],
                        start=(k == 0), stop=False,
                    )
                nc.tensor.matmul(
                    ps_t[:], ones16[:], gb_sb[:, cs], start=False, stop=True,
                )
                nc.scalar.copy(G0T[:, cs], ps_t[:])

            # hn2 = 0.5*(W_hh@h0 + b_hh)_n in the [h, b] layout
            ps_n = psrp.tile([P, 4, BS], F32, tag="gr")
            for m in range(4):
                for k in range(KO):
                    nc.tensor.matmul(
                        ps_n[:, m, :],
                        whh_sb[:, k, (m + 8) * P:(m + 9) * P],
                        h0b[:, k, :],
                        start=(k == 0), stop=(k == KO - 1),
                    )
            hn2 = const.tile([P, KO, BS], BF16, tag="hn2")
            nc.vector.scalar_tensor_tensor(
                hn2[:], ps_n[:], 0.5,
                bhhnh_sb[:, :, None].to_broadcast((P, KO, BS)),
                ALU.mult, ALU.add,
            )

            # ---- per-granule projection state ----
            stage_tiles = {}

            def emit_proj_mm(g, u):
                if u == 0:
                    stage_tiles[g] = stagep.tile(
                        [P, VPAD], FP16, tag="stage", name=f"stage{g}"
                    )
                pp = psp.tile([P, VC], F32, tag="pp", name=f"pp{g}_{u}")
                for k in range(KO):
                    nc.tensor.matmul(
                        pp[:],
                        resT[:, k, g * GR:(g + 1) * GR, :],
                        wout[:, k, u * VC:(u + 1) * VC],
                        start=(k == 0), stop=(k == KO - 1),
                    )
                return pp

            def emit_proj_tail(g, u, pp):
                # Pool engine is otherwise idle; keeping the PSUM->fp16 drains
                # off Act/DVE protects the serial GRU chain from head-of-line
                # blocking in those engine queues.
                st = stage_tiles[g]
                nc.gpsimd.tensor_scalar_add(st[:, u * VC:(u + 1) * VC], pp[:], 0.0)
                if u == NVC - 1:
                    nc.sync.dma_start(OUT[g * P:(g + 1) * P, :], st[:])
                    del stage_tiles[g]

            # ---- GRU steps ----
            for t in range(STEPS):
                psr = psrp.tile([P, 4, BS], F32, tag="gr")
                pszn = psznp.tile([P, 8, BS], F32, tag="gzn")
                for m in range(GM):
                    dst = psr[:, m, :] if m < 4 else pszn[:, m - 4, :]
                    nc.tensor.matmul(
                        dst, G0T[:, m * P:(m + 1) * P], i16_sb[:],
                        start=True, stop=False,
                    )
                    for k in range(KO):
                        rhs = x0_sb[:, k, :] if t == 0 else resT[:, k, t - 1, :]
                        nc.tensor.matmul(
                            dst, wih[:, k, m, :], rhs,
                            start=False, stop=(k == KO - 1),
                        )
                # projection units (granule g = t//GR - 1) interleave here to
                # fill the PE stream while the elementwise chain runs
                pps = []
                g = t // GR - 1
                if g >= 0:
                    for u in range(UPS * (t % GR), UPS * (t % GR) + UPS):
                        pps.append((g, u, emit_proj_mm(g, u)))

                tr = sp.tile([P, KO, BS], BF16, tag="tr")
                nc.scalar.activation(tr[:], psr[:], AF.Tanh, scale=0.5)
                tz = sp.tile([P, KO, BS], BF16, tag="tz")
                nc.scalar.activation(tz[:], pszn[:, 0:4, :], AF.Tanh, scale=0.5)
                a = sp.tile([P, KO, BS], BF16, tag="a")
                nc.vector.tensor_mul(a[:], tr[:], hn2[:])
                sn = sp.tile([P, KO, BS], BF16, tag="sn")
                nc.vector.tensor_add(sn[:], pszn[:, 4:8, :], a[:])
                n_ = sp.tile([P, KO, BS], BF16, tag="n")
                nc.scalar.activation(n_[:], sn[:], AF.Tanh, scale=1.0)
                # d = 0.5 - 0.5*tz ; c1 = h0h*(1+tz) = h0h + h0h*tz
                d = sp.tile([P, KO, BS], BF16, tag="d")
                nc.vector.tensor_scalar(d[:], tz[:], -0.5, 0.5, ALU.mult, ALU.add)
                u_ = sp.tile([P, KO, BS], BF16, tag="u")
                nc.vector.tensor_mul(u_[:], tz[:], h0h[:])
                c1 = sp.tile([P, KO, BS], BF16, tag="c1")
                nc.vector.tensor_add(c1[:], u_[:], h0h[:])
                e = sp.tile([P, KO, BS], BF16, tag="e")
                nc.vector.tensor_mul(e[:], n_[:], d[:])
                # h' = e + c1, written straight into the res history
                nc.vector.tensor_add(resT[:, :, t, :], e[:], c1[:])

                for g, u, pp in pps:
                    emit_proj_tail(g, u, pp)

            # ---- drain the last granule's projection ----
            g = NGRAN - 1
            for u in range(NVC):
                pp = emit_proj_mm(g, u)
                emit_proj_tail(g, u, pp)

    nc.compile()
    return nc


def _shard_inputs(feat, W_hp, b_hp, W_ih, W_hh, b_ih, b_hh, embed, W_out, b_out):
    bf = ml_dtypes.bfloat16
    feat = np.asarray(feat)
    WhpT = np.ascontiguousarray(np.asarray(W_hp).T, dtype=np.float32)
    WhhT = np.ascontiguousarray(np.asarray(W_hh).T).astype(bf)
    WihT = np.ascontiguousarray(np.asarray(W_ih).T).astype(bf)
    x0T = np.ascontiguousarray(
        np.repeat(np.asarray(embed)[SOS][:, None], BS, axis=1)
    ).astype(bf)
    b_ih = np.asarray(b_ih, np.float32)
    b_hh = np.asarray(b_hh, np.float32)
    gbias = np.concatenate(
        [(b_hh + b_ih)[:2 * HID], (0.5 * b_hh + b_ih)[2 * HID:]]
    ).astype(bf)
    bhhn_half = (0.5 * b_hh)[2 * HID:].copy()
    Wo = np.zeros((NVQ * VPAD, HID), np.float32)
    Wo[:VOCAB] = np.asarray(W_out)
    common = dict(
        WhpT=WhpT, WhhT=WhhT, WihT=WihT, x0T=x0T,
        b_hp=np.asarray(b_hp, np.float32),
        gbias=gbias, bhhn_half=bhhn_half,
        I16=np.eye(BS, dtype=np.float32).astype(bf),
    )
    featT_halves = [
        np.ascontiguousarray(feat[hb * BS:(hb + 1) * BS].T, dtype=np.float32)
        for hb in range(2)
    ]
    woutT_quarters = [
        np.ascontiguousarray(Wo[vq * VPAD:(vq + 1) * VPAD].T).astype(bf)
        for vq in range(NVQ)
    ]
    in_maps = []
    for c in range(NCORES):
        hb, vq = divmod(c, NVQ)
        m = dict(common)
        m["featT"] = featT_halves[hb]
        m["WoutT"] = woutT_quarters[vq]
        in_maps.append(m)
    return in_maps


def kernel(**inputs):
    global LAST_RESULTS
    args = {k: np.asarray(v) for k, v in inputs.items()}
    in_maps = _shard_inputs(
        args["feat"], args["W_hp"], args["b_hp"], args["W_ih"], args["W_hh"],
        args["b_ih"], args["b_hh"], args["embed"], args["W_out"], args["b_out"],
    )
    nc = build()
    res = run_bass_kernel_spmd(nc, in_maps, core_ids=list(range(NCORES)))
    LAST_RESULTS = res
    full = np.empty((BATCH, VOCAB, STEPS), np.float32)
    for c in range(NCORES):
        hb, vq = divmod(c, NVQ)
        v0 = vq * VPAD
        nv = min(VPAD, VOCAB - v0)
        if nv <= 0:
            continue
        # OUT is [(T*BS), VPAD] fp16, row t*BS + b
        o = np.asarray(res.results[c]["OUT"], dtype=np.float32)
        o = o.reshape(STEPS, BS, VPAD)
        full[hb * BS:(hb + 1) * BS, v0:v0 + nv, :] = (
            o[:, :, :nv].transpose(1, 2, 0)
        )
    b_out = np.asarray(args["b_out"], np.float32)
    if np.any(b_out):
        full += b_out[None, :, None]
    return np.ascontiguousarray(full, dtype=np.float32)


# revision 18
# speedup vs baseline: 2.6725x; 1.0211x over previous
"""Trainium2 Bass kernel for the GRU caption model.

Computes: h0 = feat @ W_hp.T + b_hp; 200-step GRU with constant hidden-proj
gate pre-activations; logits = outs @ W_out.T (+ b_out on host) -> [B, V, T].

Sharding: hybrid 2-way batch x 4-way vocab across the 8 cores.  Core c
handles batch half c//4 (16 rows) and vocab quarter c%4 (7680 padded rows).
Each core runs its batch half's GRU; the projection uses the GRU state tiles
as the *stationary* matmul operand ([128 h, 128 (t,b)] chunks) and streams
W_out columns, so each 8-timestep "granule" yields a [128 (t,b), 7680 v]
fp16 tile that leaves in one large DMA.  PSUM->fp16 drains run on the
otherwise-idle GPSIMD engine so Act/DVE serve only the serial GRU chain.
Gate constants (W_hh @ h0 + biases) are accumulated into the gates PSUM
through a small identity matmul; the r-gate PSUM is a separate tile so the
chain's first tanh only waits on the r matmuls.
"""

import numpy as np
import ml_dtypes

import concourse.bass as bass
import concourse.mybir as mybir
import concourse.tile as tile
from concourse import bacc
from concourse.bass_utils import run_bass_kernel_spmd

F32 = mybir.dt.float32
BF16 = mybir.dt.bfloat16
FP16 = mybir.dt.float16
AF = mybir.ActivationFunctionType
ALU = mybir.AluOpType

VOCAB = 30522
HID = 512
FEAT = 2048
STEPS = 200
BATCH = 32
SOS = 101
NCORES = 8
P = 128
KO = HID // P            # 4 h-chunks
GM = 3 * HID // P        # 12 gate row-groups (r: 0-3, z: 4-7, n: 8-11)
KF = FEAT // P           # 16 feat chunks
BS = 16                  # per-core batch shard
NVQ = 4                  # vocab quarters
VPAD = 30720 // NVQ      # per-core padded vocab rows = 7680
GR = P // BS             # granule timesteps -> 128 (t,b) columns (8)
NGRAN = STEPS // GR      # 25
VC = 480                 # proj v-chunk columns (psum bank holds <=512 f32)
NVC = VPAD // VC         # 16 units per granule
UPS = NVC // GR          # proj units emitted per step (2)

LAST_RESULTS = None  # test harness introspection


def build():
    nc = bacc.Bacc("TRN2", target_bir_lowering=False, debug=False)

    featT = nc.dram_tensor("featT", [FEAT, BS], FP16, kind="ExternalInput")
    WhpT = nc.dram_tensor("WhpT", [FEAT, HID], FP16, kind="ExternalInput")
    WhhT = nc.dram_tensor("WhhT", [HID, 3 * HID], BF16, kind="ExternalInput")
    WihT = nc.dram_tensor("WihT", [HID, 3 * HID], BF16, kind="ExternalInput")
    x0T = nc.dram_tensor("x0T", [HID, BS], BF16, kind="ExternalInput")
    WoutT = nc.dram_tensor("WoutT", [HID, VPAD], BF16, kind="ExternalInput")
    b_hp = nc.dram_tensor("b_hp", [HID], F32, kind="ExternalInput")
    # gate-constant bias row: [b_hh+b_ih (rz) | 0.5*b_hh+b_ih (n)]
    gbias = nc.dram_tensor("gbias", [3 * HID], BF16, kind="ExternalInput")
    bhhn_half = nc.dram_tensor("bhhn_half", [HID], F32, kind="ExternalInput")
    I16 = nc.dram_tensor("I16", [BS, BS], BF16, kind="ExternalInput")
    # row (t*BS + b) holds logits[b, :, t] for this core's vocab slice
    OUT = nc.dram_tensor("OUT", [STEPS * BS, VPAD], FP16, kind="ExternalOutput")

    with tile.TileContext(nc) as tc:
        with (
            tc.tile_pool(name="const", bufs=1) as const,
            tc.tile_pool(name="stage", bufs=2) as stagep,
            tc.tile_pool(name="step", bufs=3) as sp,
            tc.tile_pool(name="psr", bufs=2, space="PSUM") as psrp,
            tc.tile_pool(name="pszn", bufs=2, space="PSUM") as psznp,
            tc.tile_pool(name="psp", bufs=3, space="PSUM") as psp,
            tc.tile_pool(name="pst", bufs=1, space="PSUM") as pst,
        ):
            # ---- constants into SBUF ----
            # DMA_ENGINES serialize transfers, so order by when each tensor
            # is first needed: feat/whp (h0) -> whh (G0) -> wih/x0 (step 0);
            # the big wout load is only needed once projection starts (t>=8).
            featT_sb = const.tile([P, KF, BS], FP16, tag="featsb")
            nc.sync.dma_start(featT_sb[:], featT.rearrange("(k p) b -> p k b", p=P))
            whp_sb = const.tile([P, KF, HID], FP16, tag="whp")
            nc.sync.dma_start(whp_sb[:], WhpT.rearrange("(k p) h -> p k h", p=P))
            whh_sb = const.tile([P, KO, 3 * HID], BF16, tag="whh")
            nc.sync.dma_start(whh_sb[:], WhhT.rearrange("(k p) g -> p k g", p=P))
            wih = const.tile([P, KO, GM, P], BF16, tag="wih")
            nc.sync.dma_start(
                wih[:], WihT.rearrange("(k p) (m c) -> p k m c", p=P, c=P)
            )
            bhp_sb = const.tile([P, KO], F32, tag="bhp")
            nc.sync.dma_start(bhp_sb[:], b_hp.rearrange("(m p) -> p m", p=P))
            gb_sb = const.tile([1, 3 * HID], BF16, tag="gb")
            nc.sync.dma_start(gb_sb[:], gbias[None, :])
            bhhnh_sb = const.tile([P, KO], F32, tag="bhhnh")
            nc.sync.dma_start(bhhnh_sb[:], bhhn_half.rearrange("(m p) -> p m", p=P))
            i16_sb = const.tile([BS, BS], BF16, tag="i16")
            nc.sync.dma_start(i16_sb[:], I16[:, :])
            ones16 = const.tile([1, BS], BF16, tag="ones16")
            nc.vector.memset(ones16[:], 1.0)
            x0_sb = const.tile([P, KO, BS], BF16, tag="x0")
            nc.sync.dma_start(x0_sb[:], x0T.rearrange("(k p) b -> p k b", p=P))
            wout = const.tile([P, KO, VPAD], BF16, tag="wout")
            nc.sync.dma_start(wout[:], WoutT.rearrange("(k p) v -> p k v", p=P))

            # resT[p, k, t, b] = h_{t+1}[k*128+p, b]; (t, b) last so an
            # 8-step granule slice is a contiguous 128-column stationary
            # operand for the projection matmuls.
            resT = const.tile([P, KO, STEPS, BS], BF16, tag="resT")

            # ---- h0 = feat @ W_hp.T + b_hp (fp32, exact) ----
            ps_h = psznp.tile([P, 8, BS], F32, tag="gzn")
            for ko in range(KO):
                for kf in range(KF):
                    nc.tensor.matmul(
                        ps_h[:, ko, :],
                        whp_sb[:, kf, ko * P:(ko + 1) * P],
                        featT_sb[:, kf, :],
                        start=(kf == 0), stop=(kf == KF - 1),
                    )
            h0T = const.tile([P, KO, BS], F32, tag="h0T")
            for ko in range(KO):
                nc.scalar.activation(
                    h0T[:, ko, :], ps_h[:, ko, :], AF.Identity,
                    bias=bhp_sb[:, ko, None], scale=1.0,
                )
            h0h = const.tile([P, KO, BS], BF16, tag="h0h")
            nc.scalar.mul(h0h[:], h0T[:], 0.5)
            h0b = const.tile([P, KO, BS], BF16, tag="h0b")
            nc.scalar.copy(h0b[:], h0T[:])

            # ---- gate constants, computed directly transposed ----
            # G0T[b, g] = (h0 @ W_hh.T)[b, g] + bias[g], with the n third
            # using 0.5*h0 and the E_n bias mix (folded in via gbias and an
            # extra ones-row matmul so bias rides the PSUM accumulation).
            G0T = const.tile([BS, GM * P], BF16, tag="G0T")
            for gc in range(3):
                cs = slice(gc * 512, (gc + 1) * 512)
                lhs = h0b if gc < 2 else h0h
                ps_t = pst.tile([BS, 512], F32, tag="tp")
                for k in range(KO):
                    nc.tensor.matmul(
                        ps_t[:], lhs[:, k, :], whh_sb[:, k, cs],
                        start=(k == 0), stop=False,
                    )
                nc.tensor.matmul(
                    ps_t[:], ones16[:], gb_sb[:, cs], start=False, stop=True,
                )
                nc.scalar.copy(G0T[:, cs], ps_t[:])

            # hn2 = 0.5*(W_hh@h0 + b_hh)_n in the [h, b] layout
            ps_n = psrp.tile([P, 4, BS], F32, tag="gr")
            for m in range(4):
                for k in range(KO):
                    nc.tensor.matmul(
                        ps_n[:, m, :],
                        whh_sb[:, k, (m + 8) * P:(m + 9) * P],
                        h0b[:, k, :],
                        start=(k == 0), stop=(k == KO - 1),
                    )
            hn2 = const.tile([P, KO, BS], BF16, tag="hn2")
            nc.vector.scalar_tensor_tensor(
                hn2[:], ps_n[:], 0.5,
                bhhnh_sb[:, :, None].to_broadcast((P, KO, BS)),
                ALU.mult, ALU.add,
            )

            # ---- per-granule projection state ----
            stage_tiles = {}

            def emit_proj_mm(g, u):
                if u == 0:
                    stage_tiles[g] = stagep.tile(
                        [P, VPAD], FP16, tag="stage", name=f"stage{g}"
                    )
                pp = psp.tile([P, VC], F32, tag="pp", name=f"pp{g}_{u}")
                for k in range(KO):
                    nc.tensor.matmul(
                        pp[:],
                        resT[:, k, g * GR:(g + 1) * GR, :],
                        wout[:, k, u * VC:(u + 1) * VC],
                        start=(k == 0), stop=(k == KO - 1),
                    )
                return pp

            def emit_proj_tail(g, u, pp):
                # Pool engine is otherwise idle; keeping the PSUM->fp16 drains
                # off Act/DVE protects the serial GRU chain from head-of-line
                # blocking in those engine queues.
                st = stage_tiles[g]
                nc.gpsimd.tensor_scalar_add(st[:, u * VC:(u + 1) * VC], pp[:], 0.0)
                # two half DMAs per granule so the first half's store overlaps
                # the second half's matmuls/converts
                half = NVC // 2 * VC
                if u == NVC // 2 - 1:
                    nc.sync.dma_start(
                        OUT[g * P:(g + 1) * P, 0:half], st[:, 0:half]
                    )
                elif u == NVC - 1:
                    nc.sync.dma_start(
                        OUT[g * P:(g + 1) * P, half:VPAD], st[:, half:VPAD]
                    )
                    del stage_tiles[g]

            # ---- GRU steps ----
            for t in range(STEPS):
                psr = psrp.tile([P, 4, BS], F32, tag="gr")
                pszn = psznp.tile([P, 8, BS], F32, tag="gzn")
                for m in range(GM):
                    dst = psr[:, m, :] if m < 4 else pszn[:, m - 4, :]
                    nc.tensor.matmul(
                        dst, G0T[:, m * P:(m + 1) * P], i16_sb[:],
                        start=True, stop=False,
                    )
                    for k in range(KO):
                        rhs = x0_sb[:, k, :] if t == 0 else resT[:, k, t - 1, :]
                        nc.tensor.matmul(
                            dst, wih[:, k, m, :], rhs,
                            start=False, stop=(k == KO - 1),
                        )
                # projection units (granule g = t//GR - 1) interleave here to
                # fill the PE stream while the elementwise chain runs
                pps = []
                g = t // GR - 1
                if g >= 0:
                    for u in range(UPS * (t % GR), UPS * (t % GR) + UPS):
                        pps.append((g, u, emit_proj_mm(g, u)))

                tr = sp.tile([P, KO, BS], BF16, tag="tr")
                nc.scalar.activation(tr[:], psr[:], AF.Tanh, scale=0.5)
                tz = sp.tile([P, KO, BS], BF16, tag="tz")
                nc.scalar.activation(tz[:], pszn[:, 0:4, :], AF.Tanh, scale=0.5)
                a = sp.tile([P, KO, BS], BF16, tag="a")
                nc.vector.tensor_mul(a[:], tr[:], hn2[:])
                sn = sp.tile([P, KO, BS], BF16, tag="sn")
                nc.vector.tensor_add(sn[:], pszn[:, 4:8, :], a[:])
                n_ = sp.tile([P, KO, BS], BF16, tag="n")
                nc.scalar.activation(n_[:], sn[:], AF.Tanh, scale=1.0)
                # d = 0.5 - 0.5*tz ; c1 = h0h*(1+tz) = h0h + h0h*tz
                d = sp.tile([P, KO, BS], BF16, tag="d")
                nc.vector.tensor_scalar(d[:], tz[:], -0.5, 0.5, ALU.mult, ALU.add)
                u_ = sp.tile([P, KO, BS], BF16, tag="u")
                nc.vector.tensor_mul(u_[:], tz[:], h0h[:])
                c1 = sp.tile([P, KO, BS], BF16, tag="c1")
                nc.vector.tensor_add(c1[:], u_[:], h0h[:])
                e = sp.tile([P, KO, BS], BF16, tag="e")
                nc.vector.tensor_mul(e[:], n_[:], d[:])
                # h' = e + c1, written straight into the res history
                nc.vector.tensor_add(resT[:, :, t, :], e[:], c1[:])

                for g, u, pp in pps:
                    emit_proj_tail(g, u, pp)

            # ---- drain the last granule's projection ----
            g = NGRAN - 1
            for u in range(NVC):
                pp = emit_proj_mm(g, u)
                emit_proj_tail(g, u, pp)

    nc.compile()
    return nc


def _shard_inputs(feat, W_hp, b_hp, W_ih, W_hh, b_ih, b_hh, embed, W_out, b_out):
    bf = ml_dtypes.bfloat16
    feat = np.asarray(feat)
    WhpT = np.ascontiguousarray(np.asarray(W_hp).T, dtype=np.float16)
    WhhT = np.ascontiguousarray(np.asarray(W_hh).T).astype(bf)
    WihT = np.ascontiguousarray(np.asarray(W_ih).T).astype(bf)
    x0T = np.ascontiguousarray(
        np.repeat(np.asarray(embed)[SOS][:, None], BS, axis=1)
    ).astype(bf)
    b_ih = np.asarray(b_ih, np.float32)
    b_hh = np.asarray(b_hh, np.float32)
    gbias = np.concatenate(
        [(b_hh + b_ih)[:2 * HID], (0.5 * b_hh + b_ih)[2 * HID:]]
    ).astype(bf)
    bhhn_half = (0.5 * b_hh)[2 * HID:].copy()
    Wo = np.zeros((NVQ * VPAD, HID), np.float32)
    Wo[:VOCAB] = np.asarray(W_out)
    common = dict(
        WhpT=WhpT, WhhT=WhhT, WihT=WihT, x0T=x0T,
        b_hp=np.asarray(b_hp, np.float32),
        gbias=gbias, bhhn_half=bhhn_half,
        I16=np.eye(BS, dtype=np.float32).astype(bf),
    )
    featT_halves = [
        np.ascontiguousarray(feat[hb * BS:(hb + 1) * BS].T, dtype=np.float32)
        for hb in range(2)
    ]
    woutT_quarters = [
        np.ascontiguousarray(Wo[vq * VPAD:(vq + 1) * VPAD].T).astype(bf)
        for vq in range(NVQ)
    ]
    in_maps = []
    for c in range(NCORES):
        hb, vq = divmod(c, NVQ)
        m = dict(common)
        m["featT"] = featT_halves[hb]
        m["WoutT"] = woutT_quarters[vq]
        in_maps.append(m)
    return in_maps


def kernel(**inputs):
    global LAST_RESULTS
    args = {k: np.asarray(v) for k, v in inputs.items()}
    in_maps = _shard_inputs(
        args["feat"], args["W_hp"], args["b_hp"], args["W_ih"], args["W_hh"],
        args["b_ih"], args["b_hh"], args["embed"], args["W_out"], args["b_out"],
    )
    nc = build()
    res = run_bass_kernel_spmd(nc, in_maps, core_ids=list(range(NCORES)))
    LAST_RESULTS = res
    full = np.empty((BATCH, VOCAB, STEPS), np.float32)
    for c in range(NCORES):
        hb, vq = divmod(c, NVQ)
        v0 = vq * VPAD
        nv = min(VPAD, VOCAB - v0)
        if nv <= 0:
            continue
        # OUT is [(T*BS), VPAD] fp16, row t*BS + b
        o = np.asarray(res.results[c]["OUT"], dtype=np.float32)
        o = o.reshape(STEPS, BS, VPAD)
        full[hb * BS:(hb + 1) * BS, v0:v0 + nv, :] = (
            o[:, :, :nv].transpose(1, 2, 0)
        )
    b_out = np.asarray(args["b_out"], np.float32)
    if np.any(b_out):
        full += b_out[None, :, None]
    return np.ascontiguousarray(full, dtype=np.float32)


# revision 19
# speedup vs baseline: 2.6725x; 1.0000x over previous
"""Trainium2 Bass kernel for the GRU caption model.

Computes: h0 = feat @ W_hp.T + b_hp; 200-step GRU with constant hidden-proj
gate pre-activations; logits = outs @ W_out.T (+ b_out on host) -> [B, V, T].

Sharding: hybrid 2-way batch x 4-way vocab across the 8 cores.  Core c
handles batch half c//4 (16 rows) and vocab quarter c%4 (7680 padded rows).
Each core runs its batch half's GRU; the projection uses the GRU state tiles
as the *stationary* matmul operand ([128 h, 128 (t,b)] chunks) and streams
W_out columns, so each 8-timestep "granule" yields a [128 (t,b), 7680 v]
fp16 tile that leaves in one large DMA.  PSUM->fp16 drains run on the
otherwise-idle GPSIMD engine so Act/DVE serve only the serial GRU chain.
Gate constants (W_hh @ h0 + biases) are accumulated into the gates PSUM
through a small identity matmul; the r-gate PSUM is a separate tile so the
chain's first tanh only waits on the r matmuls.
"""

import numpy as np
import ml_dtypes

import concourse.bass as bass
import concourse.mybir as mybir
import concourse.tile as tile
from concourse import bacc
from concourse.bass_utils import run_bass_kernel_spmd

F32 = mybir.dt.float32
BF16 = mybir.dt.bfloat16
FP16 = mybir.dt.float16
AF = mybir.ActivationFunctionType
ALU = mybir.AluOpType

VOCAB = 30522
HID = 512
FEAT = 2048
STEPS = 200
BATCH = 32
SOS = 101
NCORES = 8
P = 128
KO = HID // P            # 4 h-chunks
GM = 3 * HID // P        # 12 gate row-groups (r: 0-3, z: 4-7, n: 8-11)
KF = FEAT // P           # 16 feat chunks
BS = 16                  # per-core batch shard
NVQ = 4                  # vocab quarters
VPAD = 30720 // NVQ      # per-core padded vocab rows = 7680
GR = P // BS             # granule timesteps -> 128 (t,b) columns (8)
NGRAN = STEPS // GR      # 25
VC = 480                 # proj v-chunk columns (psum bank holds <=512 f32)
NVC = VPAD // VC         # 16 units per granule
UPS = NVC // GR          # proj units emitted per step (2)

LAST_RESULTS = None  # test harness introspection


def build():
    nc = bacc.Bacc("TRN2", target_bir_lowering=False, debug=False)

    featT = nc.dram_tensor("featT", [FEAT, BS], FP16, kind="ExternalInput")
    WhpT = nc.dram_tensor("WhpT", [FEAT, HID], FP16, kind="ExternalInput")
    WhhT = nc.dram_tensor("WhhT", [HID, 3 * HID], BF16, kind="ExternalInput")
    WihT = nc.dram_tensor("WihT", [HID, 3 * HID], BF16, kind="ExternalInput")
    x0T = nc.dram_tensor("x0T", [HID, BS], BF16, kind="ExternalInput")
    WoutT = nc.dram_tensor("WoutT", [HID, VPAD], BF16, kind="ExternalInput")
    b_hp = nc.dram_tensor("b_hp", [HID], F32, kind="ExternalInput")
    # gate-constant bias row: [b_hh+b_ih (rz) | 0.5*b_hh+b_ih (n)]
    gbias = nc.dram_tensor("gbias", [3 * HID], BF16, kind="ExternalInput")
    bhhn_half = nc.dram_tensor("bhhn_half", [HID], F32, kind="ExternalInput")
    I16 = nc.dram_tensor("I16", [BS, BS], BF16, kind="ExternalInput")
    # row (t*BS + b) holds logits[b, :, t] for this core's vocab slice
    OUT = nc.dram_tensor("OUT", [STEPS * BS, VPAD], FP16, kind="ExternalOutput")

    with tile.TileContext(nc) as tc:
        with (
            tc.tile_pool(name="const", bufs=1) as const,
            tc.tile_pool(name="stage", bufs=2) as stagep,
            tc.tile_pool(name="step", bufs=3) as sp,
            tc.tile_pool(name="psr", bufs=2, space="PSUM") as psrp,
            tc.tile_pool(name="pszn", bufs=2, space="PSUM") as psznp,
            tc.tile_pool(name="psp", bufs=3, space="PSUM") as psp,
            tc.tile_pool(name="pst", bufs=1, space="PSUM") as pst,
        ):
            # ---- constants into SBUF ----
            # DMA_ENGINES serialize transfers, so order by when each tensor
            # is first needed: feat/whp (h0) -> whh (G0) -> wih/x0 (step 0);
            # the big wout load is only needed once projection starts (t>=8).
            featT_sb = const.tile([P, KF, BS], FP16, tag="featsb")
            nc.sync.dma_start(featT_sb[:], featT.rearrange("(k p) b -> p k b", p=P))
            whp_sb = const.tile([P, KF, HID], FP16, tag="whp")
            whp_src = WhpT.rearrange("(k p) h -> p k h", p=P)
            nc.sync.dma_start(whp_sb[:, 0:8, :], whp_src[:, 0:8, :])
            nc.sync.dma_start(whp_sb[:, 8:16, :], whp_src[:, 8:16, :])
            whh_sb = const.tile([P, KO, 3 * HID], BF16, tag="whh")
            whh_src = WhhT.rearrange("(k p) g -> p k g", p=P)
            for gc in range(3):
                cs = slice(gc * 512, (gc + 1) * 512)
                nc.sync.dma_start(whh_sb[:, :, cs], whh_src[:, :, cs])
            wih = const.tile([P, KO, GM, P], BF16, tag="wih")
            wih_src = WihT.rearrange("(k p) (m c) -> p k m c", p=P, c=P)
            nc.sync.dma_start(wih[:, :, 0:4, :], wih_src[:, :, 0:4, :])
            nc.sync.dma_start(wih[:, :, 4:12, :], wih_src[:, :, 4:12, :])
            bhp_sb = const.tile([P, KO], F32, tag="bhp")
            nc.sync.dma_start(bhp_sb[:], b_hp.rearrange("(m p) -> p m", p=P))
            gb_sb = const.tile([1, 3 * HID], BF16, tag="gb")
            nc.sync.dma_start(gb_sb[:], gbias[None, :])
            bhhnh_sb = const.tile([P, KO], F32, tag="bhhnh")
            nc.sync.dma_start(bhhnh_sb[:], bhhn_half.rearrange("(m p) -> p m", p=P))
            i16_sb = const.tile([BS, BS], BF16, tag="i16")
            nc.sync.dma_start(i16_sb[:], I16[:, :])
            ones16 = const.tile([1, BS], BF16, tag="ones16")
            nc.vector.memset(ones16[:], 1.0)
            x0_sb = const.tile([P, KO, BS], BF16, tag="x0")
            nc.sync.dma_start(x0_sb[:], x0T.rearrange("(k p) b -> p k b", p=P))
            wout = const.tile([P, KO, VPAD], BF16, tag="wout")
            nc.sync.dma_start(wout[:], WoutT.rearrange("(k p) v -> p k v", p=P))

            # resT[p, k, t, b] = h_{t+1}[k*128+p, b]; (t, b) last so an
            # 8-step granule slice is a contiguous 128-column stationary
            # operand for the projection matmuls.
            resT = const.tile([P, KO, STEPS, BS], BF16, tag="resT")

            # ---- h0 = feat @ W_hp.T + b_hp ----
            # two psum half-contractions so the second whp DMA half overlaps
            # the first half's matmuls
            ps_h = psznp.tile([P, 8, BS], F32, tag="gzn")
            for half in range(2):
                for ko in range(KO):
                    for kf in range(8 * half, 8 * half + 8):
                        nc.tensor.matmul(
                            ps_h[:, 4 * half + ko, :],
                            whp_sb[:, kf, ko * P:(ko + 1) * P],
                            featT_sb[:, kf, :],
                            start=(kf % 8 == 0), stop=(kf % 8 == 7),
                        )
            h0s = const.tile([P, KO, BS], F32, tag="h0s")
            nc.vector.tensor_add(h0s[:], ps_h[:, 0:4, :], ps_h[:, 4:8, :])
            h0T = const.tile([P, KO, BS], F32, tag="h0T")
            for ko in range(KO):
                nc.scalar.activation(
                    h0T[:, ko, :], h0s[:, ko, :], AF.Identity,
                    bias=bhp_sb[:, ko, None], scale=1.0,
                )
            h0h = const.tile([P, KO, BS], BF16, tag="h0h")
            nc.scalar.mul(h0h[:], h0T[:], 0.5)
            h0b = const.tile([P, KO, BS], BF16, tag="h0b")
            nc.scalar.copy(h0b[:], h0T[:])

            # ---- gate constants, computed directly transposed ----
            # G0T[b, g] = (h0 @ W_hh.T)[b, g] + bias[g], with the n third
            # using 0.5*h0 and the E_n bias mix (folded in via gbias and an
            # extra ones-row matmul so bias rides the PSUM accumulation).
            G0T = const.tile([BS, GM * P], BF16, tag="G0T")
            for gc in range(3):
                cs = slice(gc * 512, (gc + 1) * 512)
                lhs = h0b if gc < 2 else h0h
                ps_t = pst.tile([BS, 512], F32, tag="tp")
                for k in range(KO):
                    nc.tensor.matmul(
                        ps_t[:], lhs[:, k, :], whh_sb[:, k, cs],
                        start=(k == 0), stop=False,
                    )
                nc.tensor.matmul(
                    ps_t[:], ones16[:], gb_sb[:, cs], start=False, stop=True,
                )
                nc.scalar.copy(G0T[:, cs], ps_t[:])

            # hn2 = 0.5*(W_hh@h0 + b_hh)_n in the [h, b] layout
            ps_n = psrp.tile([P, 4, BS], F32, tag="gr")
            for m in range(4):
                for k in range(KO):
                    nc.tensor.matmul(
                        ps_n[:, m, :],
                        whh_sb[:, k, (m + 8) * P:(m + 9) * P],
                        h0b[:, k, :],
                        start=(k == 0), stop=(k == KO - 1),
                    )
            hn2 = const.tile([P, KO, BS], BF16, tag="hn2")
            nc.vector.scalar_tensor_tensor(
                hn2[:], ps_n[:], 0.5,
                bhhnh_sb[:, :, None].to_broadcast((P, KO, BS)),
                ALU.mult, ALU.add,
            )

            # ---- per-granule projection state ----
            stage_tiles = {}

            def emit_proj_mm(g, u):
                if u == 0:
                    stage_tiles[g] = stagep.tile(
                        [P, VPAD], FP16, tag="stage", name=f"stage{g}"
                    )
                pp = psp.tile([P, VC], F32, tag="pp", name=f"pp{g}_{u}")
                for k in range(KO):
                    nc.tensor.matmul(
                        pp[:],
                        resT[:, k, g * GR:(g + 1) * GR, :],
                        wout[:, k, u * VC:(u + 1) * VC],
                        start=(k == 0), stop=(k == KO - 1),
                    )
                return pp

            def emit_proj_tail(g, u, pp):
                # Pool engine is otherwise idle; keeping the PSUM->fp16 drains
                # off Act/DVE protects the serial GRU chain from head-of-line
                # blocking in those engine queues.
                st = stage_tiles[g]
                nc.gpsimd.tensor_scalar_add(st[:, u * VC:(u + 1) * VC], pp[:], 0.0)
                # two half DMAs per granule so the first half's store overlaps
                # the second half's matmuls/converts
                half = NVC // 2 * VC
                if u == NVC // 2 - 1:
                    nc.sync.dma_start(
                        OUT[g * P:(g + 1) * P, 0:half], st[:, 0:half]
                    )
                elif u == NVC - 1:
                    nc.sync.dma_start(
                        OUT[g * P:(g + 1) * P, half:VPAD], st[:, half:VPAD]
                    )
                    del stage_tiles[g]

            # ---- GRU steps ----
            for t in range(STEPS):
                psr = psrp.tile([P, 4, BS], F32, tag="gr")
                pszn = psznp.tile([P, 8, BS], F32, tag="gzn")
                for m in range(GM):
                    dst = psr[:, m, :] if m < 4 else pszn[:, m - 4, :]
                    nc.tensor.matmul(
                        dst, G0T[:, m * P:(m + 1) * P], i16_sb[:],
                        start=True, stop=False,
                    )
                    for k in range(KO):
                        rhs = x0_sb[:, k, :] if t == 0 else resT[:, k, t - 1, :]
                        nc.tensor.matmul(
                            dst, wih[:, k, m, :], rhs,
                            start=False, stop=(k == KO - 1),
                        )
                # projection units (granule g = t//GR - 1) interleave here to
                # fill the PE stream while the elementwise chain runs
                pps = []
                g = t // GR - 1
                if g >= 0:
                    for u in range(UPS * (t % GR), UPS * (t % GR) + UPS):
                        pps.append((g, u, emit_proj_mm(g, u)))

                tr = sp.tile([P, KO, BS], BF16, tag="tr")
                nc.scalar.activation(tr[:], psr[:], AF.Tanh, scale=0.5)
                tz = sp.tile([P, KO, BS], BF16, tag="tz")
                nc.scalar.activation(tz[:], pszn[:, 0:4, :], AF.Tanh, scale=0.5)
                a = sp.tile([P, KO, BS], BF16, tag="a")
                nc.vector.tensor_mul(a[:], tr[:], hn2[:])
                # Pool drains the n-gate psum so DVE's add stays all-bf16
                pn = sp.tile([P, KO, BS], BF16, tag="pn")
                nc.gpsimd.tensor_scalar_add(pn[:], pszn[:, 4:8, :], 0.0)
                sn = sp.tile([P, KO, BS], BF16, tag="sn")
                nc.vector.tensor_add(sn[:], pn[:], a[:])
                n_ = sp.tile([P, KO, BS], BF16, tag="n")
                nc.scalar.activation(n_[:], sn[:], AF.Tanh, scale=1.0)
                # d = 0.5 - 0.5*tz ; c1 = h0h*(1+tz) = h0h + h0h*tz
                d = sp.tile([P, KO, BS], BF16, tag="d")
                nc.vector.tensor_scalar(d[:], tz[:], -0.5, 0.5, ALU.mult, ALU.add)
                u_ = sp.tile([P, KO, BS], BF16, tag="u")
                nc.vector.tensor_mul(u_[:], tz[:], h0h[:])
                c1 = sp.tile([P, KO, BS], BF16, tag="c1")
                nc.vector.tensor_add(c1[:], u_[:], h0h[:])
                e = sp.tile([P, KO, BS], BF16, tag="e")
                nc.vector.tensor_mul(e[:], n_[:], d[:])
                # h' = e + c1, written straight into the res history
                nc.vector.tensor_add(resT[:, :, t, :], e[:], c1[:])

                for g, u, pp in pps:
                    emit_proj_tail(g, u, pp)

            # ---- drain the last granule's projection ----
            g = NGRAN - 1
            for u in range(NVC):
                pp = emit_proj_mm(g, u)
                emit_proj_tail(g, u, pp)

    nc.compile()
    return nc


def _shard_inputs(feat, W_hp, b_hp, W_ih, W_hh, b_ih, b_hh, embed, W_out, b_out):
    bf = ml_dtypes.bfloat16
    feat = np.asarray(feat)
    WhpT = np.ascontiguousarray(np.asarray(W_hp).T, dtype=np.float16)
    WhhT = np.ascontiguousarray(np.asarray(W_hh).T).astype(bf)
    WihT = np.ascontiguousarray(np.asarray(W_ih).T).astype(bf)
    x0T = np.ascontiguousarray(
        np.repeat(np.asarray(embed)[SOS][:, None], BS, axis=1)
    ).astype(bf)
    b_ih = np.asarray(b_ih, np.float32)
    b_hh = np.asarray(b_hh, np.float32)
    gbias = np.concatenate(
        [(b_hh + b_ih)[:2 * HID], (0.5 * b_hh + b_ih)[2 * HID:]]
    ).astype(bf)
    bhhn_half = (0.5 * b_hh)[2 * HID:].copy()
    Wo = np.zeros((NVQ * VPAD, HID), np.float32)
    Wo[:VOCAB] = np.asarray(W_out)
    common = dict(
        WhpT=WhpT, WhhT=WhhT, WihT=WihT, x0T=x0T,
        b_hp=np.asarray(b_hp, np.float32),
        gbias=gbias, bhhn_half=bhhn_half,
        I16=np.eye(BS, dtype=np.float32).astype(bf),
    )
    featT_halves = [
        np.ascontiguousarray(feat[hb * BS:(hb + 1) * BS].T, dtype=np.float32)
        for hb in range(2)
    ]
    woutT_quarters = [
        np.ascontiguousarray(Wo[vq * VPAD:(vq + 1) * VPAD].T).astype(bf)
        for vq in range(NVQ)
    ]
    in_maps = []
    for c in range(NCORES):
        hb, vq = divmod(c, NVQ)
        m = dict(common)
        m["featT"] = featT_halves[hb]
        m["WoutT"] = woutT_quarters[vq]
        in_maps.append(m)
    return in_maps


def kernel(**inputs):
    global LAST_RESULTS
    args = {k: np.asarray(v) for k, v in inputs.items()}
    in_maps = _shard_inputs(
        args["feat"], args["W_hp"], args["b_hp"], args["W_ih"], args["W_hh"],
        args["b_ih"], args["b_hh"], args["embed"], args["W_out"], args["b_out"],
    )
    nc = build()
    res = run_bass_kernel_spmd(nc, in_maps, core_ids=list(range(NCORES)))
    LAST_RESULTS = res
    full = np.empty((BATCH, VOCAB, STEPS), np.float32)
    for c in range(NCORES):
        hb, vq = divmod(c, NVQ)
        v0 = vq * VPAD
        nv = min(VPAD, VOCAB - v0)
        if nv <= 0:
            continue
        # OUT is [(T*BS), VPAD] fp16, row t*BS + b
        o = np.asarray(res.results[c]["OUT"], dtype=np.float32)
        o = o.reshape(STEPS, BS, VPAD)
        full[hb * BS:(hb + 1) * BS, v0:v0 + nv, :] = (
            o[:, :, :nv].transpose(1, 2, 0)
        )
    b_out = np.asarray(args["b_out"], np.float32)
    if np.any(b_out):
        full += b_out[None, :, None]
    return np.ascontiguousarray(full, dtype=np.float32)


# revision 20
# speedup vs baseline: 2.7836x; 1.0416x over previous
"""Trainium2 Bass kernel for the GRU caption model.

Computes: h0 = feat @ W_hp.T + b_hp; 200-step GRU with constant hidden-proj
gate pre-activations; logits = outs @ W_out.T (+ b_out on host) -> [B, V, T].

Sharding: hybrid 2-way batch x 4-way vocab across the 8 cores.  Core c
handles batch half c//4 (16 rows) and vocab quarter c%4 (7680 padded rows).
Each core runs its batch half's GRU; the projection uses the GRU state tiles
as the *stationary* matmul operand ([128 h, 128 (t,b)] chunks) and streams
W_out columns, so each 8-timestep "granule" yields a [128 (t,b), 7680 v]
fp16 tile that leaves in one large DMA.  PSUM->fp16 drains run on the
otherwise-idle GPSIMD engine so Act/DVE serve only the serial GRU chain.
Gate constants (W_hh @ h0 + biases) are accumulated into the gates PSUM
through a small identity matmul; the r-gate PSUM is a separate tile so the
chain's first tanh only waits on the r matmuls.
"""

import numpy as np
import ml_dtypes

import concourse.bass as bass
import concourse.mybir as mybir
import concourse.tile as tile
from concourse import bacc
from concourse.bass_utils import run_bass_kernel_spmd

F32 = mybir.dt.float32
BF16 = mybir.dt.bfloat16
FP16 = mybir.dt.float16
AF = mybir.ActivationFunctionType
ALU = mybir.AluOpType

VOCAB = 30522
HID = 512
FEAT = 2048
STEPS = 200
BATCH = 32
SOS = 101
NCORES = 8
P = 128
KO = HID // P            # 4 h-chunks
GM = 3 * HID // P        # 12 gate row-groups (r: 0-3, z: 4-7, n: 8-11)
KF = FEAT // P           # 16 feat chunks
BS = 16                  # per-core batch shard
NVQ = 4                  # vocab quarters
VPAD = 30720 // NVQ      # per-core padded vocab rows = 7680
GR = P // BS             # granule timesteps -> 128 (t,b) columns (8)
NGRAN = STEPS // GR      # 25
VC = 480                 # proj v-chunk columns (psum bank holds <=512 f32)
NVC = VPAD // VC         # 16 units per granule
UPS = NVC // GR          # proj units emitted per step (2)

LAST_RESULTS = None  # test harness introspection


def build():
    nc = bacc.Bacc("TRN2", target_bir_lowering=False, debug=False)

    featT = nc.dram_tensor("featT", [FEAT, BS], FP16, kind="ExternalInput")
    WhpT = nc.dram_tensor("WhpT", [FEAT, HID], FP16, kind="ExternalInput")
    WhhT = nc.dram_tensor("WhhT", [HID, 3 * HID], BF16, kind="ExternalInput")
    WihT = nc.dram_tensor("WihT", [HID, 3 * HID], BF16, kind="ExternalInput")
    x0T = nc.dram_tensor("x0T", [HID, BS], BF16, kind="ExternalInput")
    WoutT = nc.dram_tensor("WoutT", [HID, VPAD], BF16, kind="ExternalInput")
    b_hp = nc.dram_tensor("b_hp", [HID], F32, kind="ExternalInput")
    # gate-constant bias row: [b_hh+b_ih (rz) | 0.5*b_hh+b_ih (n)]
    gbias = nc.dram_tensor("gbias", [3 * HID], BF16, kind="ExternalInput")
    bhhn_half = nc.dram_tensor("bhhn_half", [HID], F32, kind="ExternalInput")
    I16 = nc.dram_tensor("I16", [BS, BS], BF16, kind="ExternalInput")
    # row (t*BS + b) holds logits[b, :, t] for this core's vocab slice
    OUT = nc.dram_tensor("OUT", [STEPS * BS, VPAD], FP16, kind="ExternalOutput")

    with tile.TileContext(nc) as tc:
        with (
            tc.tile_pool(name="const", bufs=1) as const,
            tc.tile_pool(name="stage", bufs=2) as stagep,
            tc.tile_pool(name="step", bufs=3) as sp,
            tc.tile_pool(name="psr", bufs=2, space="PSUM") as psrp,
            tc.tile_pool(name="pszn", bufs=2, space="PSUM") as psznp,
            tc.tile_pool(name="psp", bufs=3, space="PSUM") as psp,
            tc.tile_pool(name="pst", bufs=1, space="PSUM") as pst,
        ):
            # ---- constants into SBUF ----
            # DMA_ENGINES serialize transfers, so order by when each tensor
            # is first needed: feat/whp (h0) -> whh (G0) -> wih/x0 (step 0);
            # the big wout load is only needed once projection starts (t>=8).
            featT_sb = const.tile([P, KF, BS], FP16, tag="featsb")
            nc.sync.dma_start(featT_sb[:], featT.rearrange("(k p) b -> p k b", p=P))
            whp_sb = const.tile([P, KF, HID], FP16, tag="whp")
            whp_src = WhpT.rearrange("(k p) h -> p k h", p=P)
            nc.sync.dma_start(whp_sb[:, 0:8, :], whp_src[:, 0:8, :])
            nc.sync.dma_start(whp_sb[:, 8:16, :], whp_src[:, 8:16, :])
            whh_sb = const.tile([P, KO, 3 * HID], BF16, tag="whh")
            whh_src = WhhT.rearrange("(k p) g -> p k g", p=P)
            for gc in range(3):
                cs = slice(gc * 512, (gc + 1) * 512)
                nc.sync.dma_start(whh_sb[:, :, cs], whh_src[:, :, cs])
            wih = const.tile([P, KO, GM, P], BF16, tag="wih")
            wih_src = WihT.rearrange("(k p) (m c) -> p k m c", p=P, c=P)
            nc.sync.dma_start(wih[:, :, 0:4, :], wih_src[:, :, 0:4, :])
            nc.sync.dma_start(wih[:, :, 4:12, :], wih_src[:, :, 4:12, :])
            # small constants ride the Activation engine's DMA queue so
            # their fixed per-DMA overheads overlap SP's big weight loads
            bhp_sb = const.tile([P, KO], F32, tag="bhp")
            nc.scalar.dma_start(bhp_sb[:], b_hp.rearrange("(m p) -> p m", p=P))
            gb_sb = const.tile([1, 3 * HID], BF16, tag="gb")
            nc.scalar.dma_start(gb_sb[:], gbias[None, :])
            bhhnh_sb = const.tile([P, KO], F32, tag="bhhnh")
            nc.scalar.dma_start(bhhnh_sb[:], bhhn_half.rearrange("(m p) -> p m", p=P))
            i16_sb = const.tile([BS, BS], BF16, tag="i16")
            nc.scalar.dma_start(i16_sb[:], I16[:, :])
            ones16 = const.tile([1, BS], BF16, tag="ones16")
            nc.vector.memset(ones16[:], 1.0)
            x0_sb = const.tile([P, KO, BS], BF16, tag="x0")
            nc.scalar.dma_start(x0_sb[:], x0T.rearrange("(k p) b -> p k b", p=P))
            # wout in v-chunks: projection unit u only needs its own chunk,
            # so granule 0 can start before the whole 7.9MB lands
            wout = const.tile([P, KO, VPAD], BF16, tag="wout")
            wout_src = WoutT.rearrange("(k p) v -> p k v", p=P)
            WCH = VPAD // 4
            for wc in range(4):
                ws = slice(wc * WCH, (wc + 1) * WCH)
                nc.sync.dma_start(wout[:, :, ws], wout_src[:, :, ws])

            # resT[p, k, t, b] = h_{t+1}[k*128+p, b]; (t, b) last so an
            # 8-step granule slice is a contiguous 128-column stationary
            # operand for the projection matmuls.
            resT = const.tile([P, KO, STEPS, BS], BF16, tag="resT")

            # ---- h0 = feat @ W_hp.T + b_hp ----
            # two psum half-contractions so the second whp DMA half overlaps
            # the first half's matmuls
            ps_h = psznp.tile([P, 8, BS], F32, tag="gzn")
            for half in range(2):
                for ko in range(KO):
                    for kf in range(8 * half, 8 * half + 8):
                        nc.tensor.matmul(
                            ps_h[:, 4 * half + ko, :],
                            whp_sb[:, kf, ko * P:(ko + 1) * P],
                            featT_sb[:, kf, :],
                            start=(kf % 8 == 0), stop=(kf % 8 == 7),
                        )
            h0s = const.tile([P, KO, BS], F32, tag="h0s")
            nc.vector.tensor_add(h0s[:], ps_h[:, 0:4, :], ps_h[:, 4:8, :])
            h0T = const.tile([P, KO, BS], F32, tag="h0T")
            for ko in range(KO):
                nc.scalar.activation(
                    h0T[:, ko, :], h0s[:, ko, :], AF.Identity,
                    bias=bhp_sb[:, ko, None], scale=1.0,
                )
            h0h = const.tile([P, KO, BS], BF16, tag="h0h")
            nc.scalar.mul(h0h[:], h0T[:], 0.5)
            h0b = const.tile([P, KO, BS], BF16, tag="h0b")
            nc.scalar.copy(h0b[:], h0T[:])

            # ---- gate constants, computed directly transposed ----
            # G0T[b, g] = (h0 @ W_hh.T)[b, g] + bias[g], with the n third
            # using 0.5*h0 and the E_n bias mix (folded in via gbias and an
            # extra ones-row matmul so bias rides the PSUM accumulation).
            G0T = const.tile([BS, GM * P], BF16, tag="G0T")
            for gc in range(3):
                cs = slice(gc * 512, (gc + 1) * 512)
                lhs = h0b if gc < 2 else h0h
                ps_t = pst.tile([BS, 512], F32, tag="tp")
                for k in range(KO):
                    nc.tensor.matmul(
                        ps_t[:], lhs[:, k, :], whh_sb[:, k, cs],
                        start=(k == 0), stop=False,
                    )
                nc.tensor.matmul(
                    ps_t[:], ones16[:], gb_sb[:, cs], start=False, stop=True,
                )
                nc.scalar.copy(G0T[:, cs], ps_t[:])

            # hn2 = 0.5*(W_hh@h0 + b_hh)_n in the [h, b] layout
            ps_n = psrp.tile([P, 4, BS], F32, tag="gr")
            for m in range(4):
                for k in range(KO):
                    nc.tensor.matmul(
                        ps_n[:, m, :],
                        whh_sb[:, k, (m + 8) * P:(m + 9) * P],
                        h0b[:, k, :],
                        start=(k == 0), stop=(k == KO - 1),
                    )
            hn2 = const.tile([P, KO, BS], BF16, tag="hn2")
            nc.vector.scalar_tensor_tensor(
                hn2[:], ps_n[:], 0.5,
                bhhnh_sb[:, :, None].to_broadcast((P, KO, BS)),
                ALU.mult, ALU.add,
            )

            # ---- per-granule projection state ----
            stage_tiles = {}

            def emit_proj_mm(g, u):
                if u == 0:
                    stage_tiles[g] = stagep.tile(
                        [P, VPAD], FP16, tag="stage", name=f"stage{g}"
                    )
                pp = psp.tile([P, VC], F32, tag="pp", name=f"pp{g}_{u}")
                for k in range(KO):
                    nc.tensor.matmul(
                        pp[:],
                        resT[:, k, g * GR:(g + 1) * GR, :],
                        wout[:, k, u * VC:(u + 1) * VC],
                        start=(k == 0), stop=(k == KO - 1),
                    )
                return pp

            def emit_proj_tail(g, u, pp):
                # Pool engine is otherwise idle; keeping the PSUM->fp16 drains
                # off Act/DVE protects the serial GRU chain from head-of-line
                # blocking in those engine queues.
                st = stage_tiles[g]
                nc.gpsimd.tensor_scalar_add(st[:, u * VC:(u + 1) * VC], pp[:], 0.0)
                # two half DMAs per granule so the first half's store overlaps
                # the second half's matmuls/converts
                half = NVC // 2 * VC
                if u == NVC // 2 - 1:
                    nc.sync.dma_start(
                        OUT[g * P:(g + 1) * P, 0:half], st[:, 0:half]
                    )
                elif u == NVC - 1:
                    nc.sync.dma_start(
                        OUT[g * P:(g + 1) * P, half:VPAD], st[:, half:VPAD]
                    )
                    del stage_tiles[g]

            # ---- GRU steps ----
            for t in range(STEPS):
                psr = psrp.tile([P, 4, BS], F32, tag="gr")
                pszn = psznp.tile([P, 8, BS], F32, tag="gzn")
                for m in range(GM):
                    dst = psr[:, m, :] if m < 4 else pszn[:, m - 4, :]
                    nc.tensor.matmul(
                        dst, G0T[:, m * P:(m + 1) * P], i16_sb[:],
                        start=True, stop=False,
                    )
                    for k in range(KO):
                        rhs = x0_sb[:, k, :] if t == 0 else resT[:, k, t - 1, :]
                        nc.tensor.matmul(
                            dst, wih[:, k, m, :], rhs,
                            start=False, stop=(k == KO - 1),
                        )
                # projection units (granule g = t//GR - 1) interleave here to
                # fill the PE stream while the elementwise chain runs
                pps = []
                g = t // GR - 1
                if g >= 0:
                    for u in range(UPS * (t % GR), UPS * (t % GR) + UPS):
                        pps.append((g, u, emit_proj_mm(g, u)))

                tr = sp.tile([P, KO, BS], BF16, tag="tr")
                nc.scalar.activation(tr[:], psr[:], AF.Tanh, scale=0.5)
                tz = sp.tile([P, KO, BS], BF16, tag="tz")
                nc.scalar.activation(tz[:], pszn[:, 0:4, :], AF.Tanh, scale=0.5)
                a = sp.tile([P, KO, BS], BF16, tag="a")
                nc.vector.tensor_mul(a[:], tr[:], hn2[:])
                # Pool drains the n-gate psum so DVE's add stays all-bf16
                pn = sp.tile([P, KO, BS], BF16, tag="pn")
                nc.gpsimd.tensor_scalar_add(pn[:], pszn[:, 4:8, :], 0.0)
                sn = sp.tile([P, KO, BS], BF16, tag="sn")
                nc.vector.tensor_add(sn[:], pn[:], a[:])
                n_ = sp.tile([P, KO, BS], BF16, tag="n")
                nc.scalar.activation(n_[:], sn[:], AF.Tanh, scale=1.0)
                # d = 0.5 - 0.5*tz ; c1 = h0h*(1+tz) = h0h + h0h*tz
                d = sp.tile([P, KO, BS], BF16, tag="d")
                nc.vector.tensor_scalar(d[:], tz[:], -0.5, 0.5, ALU.mult, ALU.add)
                u_ = sp.tile([P, KO, BS], BF16, tag="u")
                nc.vector.tensor_mul(u_[:], tz[:], h0h[:])
                c1 = sp.tile([P, KO, BS], BF16, tag="c1")
                nc.vector.tensor_add(c1[:], u_[:], h0h[:])
                e = sp.tile([P, KO, BS], BF16, tag="e")
                nc.vector.tensor_mul(e[:], n_[:], d[:])
                # h' = e + c1, written straight into the res history
                nc.vector.tensor_add(resT[:, :, t, :], e[:], c1[:])

                for g, u, pp in pps:
                    emit_proj_tail(g, u, pp)

            # ---- drain the last granule's projection ----
            g = NGRAN - 1
            for u in range(NVC):
                pp = emit_proj_mm(g, u)
                emit_proj_tail(g, u, pp)

    nc.compile()
    return nc


def _shard_inputs(feat, W_hp, b_hp, W_ih, W_hh, b_ih, b_hh, embed, W_out, b_out):
    bf = ml_dtypes.bfloat16
    feat = np.asarray(feat)
    WhpT = np.ascontiguousarray(np.asarray(W_hp).T, dtype=np.float16)
    WhhT = np.ascontiguousarray(np.asarray(W_hh).T).astype(bf)
    WihT = np.ascontiguousarray(np.asarray(W_ih).T).astype(bf)
    x0T = np.ascontiguousarray(
        np.repeat(np.asarray(embed)[SOS][:, None], BS, axis=1)
    ).astype(bf)
    b_ih = np.asarray(b_ih, np.float32)
    b_hh = np.asarray(b_hh, np.float32)
    gbias = np.concatenate(
        [(b_hh + b_ih)[:2 * HID], (0.5 * b_hh + b_ih)[2 * HID:]]
    ).astype(bf)
    bhhn_half = (0.5 * b_hh)[2 * HID:].copy()
    Wo = np.zeros((NVQ * VPAD, HID), np.float32)
    Wo[:VOCAB] = np.asarray(W_out)
    common = dict(
        WhpT=WhpT, WhhT=WhhT, WihT=WihT, x0T=x0T,
        b_hp=np.asarray(b_hp, np.float32),
        gbias=gbias, bhhn_half=bhhn_half,
        I16=np.eye(BS, dtype=np.float32).astype(bf),
    )
    featT_halves = [
        np.ascontiguousarray(feat[hb * BS:(hb + 1) * BS].T, dtype=np.float32)
        for hb in range(2)
    ]
    woutT_quarters = [
        np.ascontiguousarray(Wo[vq * VPAD:(vq + 1) * VPAD].T).astype(bf)
        for vq in range(NVQ)
    ]
    in_maps = []
    for c in range(NCORES):
        hb, vq = divmod(c, NVQ)
        m = dict(common)
        m["featT"] = featT_halves[hb]
        m["WoutT"] = woutT_quarters[vq]
        in_maps.append(m)
    return in_maps


def kernel(**inputs):
    global LAST_RESULTS
    args = {k: np.asarray(v) for k, v in inputs.items()}
    in_maps = _shard_inputs(
        args["feat"], args["W_hp"], args["b_hp"], args["W_ih"], args["W_hh"],
        args["b_ih"], args["b_hh"], args["embed"], args["W_out"], args["b_out"],
    )
    nc = build()
    res = run_bass_kernel_spmd(nc, in_maps, core_ids=list(range(NCORES)))
    LAST_RESULTS = res
    full = np.empty((BATCH, VOCAB, STEPS), np.float32)
    for c in range(NCORES):
        hb, vq = divmod(c, NVQ)
        v0 = vq * VPAD
        nv = min(VPAD, VOCAB - v0)
        if nv <= 0:
            continue
        # OUT is [(T*BS), VPAD] fp16, row t*BS + b
        o = np.asarray(res.results[c]["OUT"], dtype=np.float32)
        o = o.reshape(STEPS, BS, VPAD)
        full[hb * BS:(hb + 1) * BS, v0:v0 + nv, :] = (
            o[:, :, :nv].transpose(1, 2, 0)
        )
    b_out = np.asarray(args["b_out"], np.float32)
    if np.any(b_out):
        full += b_out[None, :, None]
    return np.ascontiguousarray(full, dtype=np.float32)


# revision 21
# speedup vs baseline: 2.8828x; 1.0356x over previous
"""Trainium2 Bass kernel for the GRU caption model.

Computes: h0 = feat @ W_hp.T + b_hp; 200-step GRU with constant hidden-proj
gate pre-activations; logits = outs @ W_out.T (+ b_out on host) -> [B, V, T].

Sharding: hybrid 2-way batch x 4-way vocab across the 8 cores.  Core c
handles batch half c//4 (16 rows) and vocab quarter c%4 (7680 padded rows).
Each core runs its batch half's GRU; the projection uses the GRU state tiles
as the *stationary* matmul operand ([128 h, 128 (t,b)] chunks) and streams
W_out columns, so each 8-timestep "granule" yields a [128 (t,b), 7680 v]
fp16 tile that leaves in one large DMA.  PSUM->fp16 drains run on the
otherwise-idle GPSIMD engine so Act/DVE serve only the serial GRU chain.
Gate constants (W_hh @ h0 + biases) are accumulated into the gates PSUM
through a small identity matmul; the r-gate PSUM is a separate tile so the
chain's first tanh only waits on the r matmuls.
"""

import numpy as np
import ml_dtypes

import concourse.bass as bass
import concourse.mybir as mybir
import concourse.tile as tile
from concourse import bacc
from concourse.bass_utils import run_bass_kernel_spmd

F32 = mybir.dt.float32
BF16 = mybir.dt.bfloat16
FP16 = mybir.dt.float16
AF = mybir.ActivationFunctionType
ALU = mybir.AluOpType

VOCAB = 30522
HID = 512
FEAT = 2048
STEPS = 200
BATCH = 32
SOS = 101
NCORES = 8
P = 128
KO = HID // P            # 4 h-chunks
GM = 3 * HID // P        # 12 gate row-groups (r: 0-3, z: 4-7, n: 8-11)
KF = FEAT // P           # 16 feat chunks
BS = 16                  # per-core batch shard
NVQ = 4                  # vocab quarters
VPAD = 30720 // NVQ      # per-core padded vocab rows = 7680
GR = P // BS             # granule timesteps -> 128 (t,b) columns (8)
NGRAN = STEPS // GR      # 25
VC = 480                 # proj v-chunk columns (psum bank holds <=512 f32)
NVC = VPAD // VC         # 16 units per granule
UPS = NVC // GR          # proj units emitted per step (2)

LAST_RESULTS = None  # test harness introspection


def build():
    nc = bacc.Bacc("TRN2", target_bir_lowering=False, debug=False)

    featT = nc.dram_tensor("featT", [FEAT, BS], FP16, kind="ExternalInput")
    WhpT = nc.dram_tensor("WhpT", [FEAT, HID], FP16, kind="ExternalInput")
    WhhT = nc.dram_tensor("WhhT", [HID, 3 * HID], BF16, kind="ExternalInput")
    WihT = nc.dram_tensor("WihT", [HID, 3 * HID], BF16, kind="ExternalInput")
    x0T = nc.dram_tensor("x0T", [HID, BS], BF16, kind="ExternalInput")
    WoutT = nc.dram_tensor("WoutT", [HID, VPAD], BF16, kind="ExternalInput")
    b_hp = nc.dram_tensor("b_hp", [HID], F32, kind="ExternalInput")
    bsum_rz = nc.dram_tensor("bsum_rz", [2 * HID], F32, kind="ExternalInput")
    bmix_n = nc.dram_tensor("bmix_n", [HID], F32, kind="ExternalInput")
    bhhn_half = nc.dram_tensor("bhhn_half", [HID], F32, kind="ExternalInput")
    # row (t*BS + b) holds logits[b, :, t] for this core's vocab slice
    OUT = nc.dram_tensor("OUT", [STEPS * BS, VPAD], FP16, kind="ExternalOutput")

    with tile.TileContext(nc) as tc:
        with (
            tc.tile_pool(name="const", bufs=1) as const,
            tc.tile_pool(name="stage", bufs=2) as stagep,
            tc.tile_pool(name="step", bufs=3) as sp,
            tc.tile_pool(name="psr", bufs=2, space="PSUM") as psrp,
            tc.tile_pool(name="pszn", bufs=2, space="PSUM") as psznp,
            tc.tile_pool(name="psp", bufs=4, space="PSUM") as psp,
        ):
            # ---- constants into SBUF ----
            # DMA_ENGINES serialize transfers, so order by when each tensor
            # is first needed: feat/whp (h0) -> whh (G0) -> wih/x0 (step 0);
            # the big wout load is only needed once projection starts (t>=8).
            featT_sb = const.tile([P, KF, BS], FP16, tag="featsb")
            nc.sync.dma_start(featT_sb[:], featT.rearrange("(k p) b -> p k b", p=P))
            whp_sb = const.tile([P, KF, HID], FP16, tag="whp")
            whp_src = WhpT.rearrange("(k p) h -> p k h", p=P)
            nc.sync.dma_start(whp_sb[:, 0:8, :], whp_src[:, 0:8, :])
            nc.sync.dma_start(whp_sb[:, 8:16, :], whp_src[:, 8:16, :])
            whh_sb = const.tile([P, KO, 3 * HID], BF16, tag="whh")
            whh_src = WhhT.rearrange("(k p) g -> p k g", p=P)
            for gc in range(3):
                cs = slice(gc * 512, (gc + 1) * 512)
                nc.sync.dma_start(whh_sb[:, :, cs], whh_src[:, :, cs])
            wih = const.tile([P, KO, GM, P], BF16, tag="wih")
            wih_src = WihT.rearrange("(k p) (m c) -> p k m c", p=P, c=P)
            nc.sync.dma_start(wih[:, :, 0:4, :], wih_src[:, :, 0:4, :])
            nc.sync.dma_start(wih[:, :, 4:12, :], wih_src[:, :, 4:12, :])
            # small constants ride the Activation engine's DMA queue so
            # their fixed per-DMA overheads overlap SP's big weight loads
            bhp_sb = const.tile([P, KO], F32, tag="bhp")
            nc.scalar.dma_start(bhp_sb[:], b_hp.rearrange("(m p) -> p m", p=P))
            bsrz_sb = const.tile([P, 8], F32, tag="bsrz")
            nc.scalar.dma_start(bsrz_sb[:], bsum_rz.rearrange("(m p) -> p m", p=P))
            bmixn_sb = const.tile([P, KO], F32, tag="bmixn")
            nc.scalar.dma_start(bmixn_sb[:], bmix_n.rearrange("(m p) -> p m", p=P))
            bhhnh_sb = const.tile([P, KO], F32, tag="bhhnh")
            nc.scalar.dma_start(bhhnh_sb[:], bhhn_half.rearrange("(m p) -> p m", p=P))
            x0_sb = const.tile([P, KO, BS], BF16, tag="x0")
            nc.scalar.dma_start(x0_sb[:], x0T.rearrange("(k p) b -> p k b", p=P))
            # wout in v-chunks: projection unit u only needs its own chunk,
            # so granule 0 can start before the whole 7.9MB lands
            wout = const.tile([P, KO, VPAD], BF16, tag="wout")
            wout_src = WoutT.rearrange("(k p) v -> p k v", p=P)
            WCH = VPAD // 4
            for wc in range(4):
                ws = slice(wc * WCH, (wc + 1) * WCH)
                nc.sync.dma_start(wout[:, :, ws], wout_src[:, :, ws])

            # resT[p, k, t, b] = h_{t+1}[k*128+p, b]; (t, b) last so an
            # 8-step granule slice is a contiguous 128-column stationary
            # operand for the projection matmuls.
            resT = const.tile([P, KO, STEPS, BS], BF16, tag="resT")

            # ---- h0 = feat @ W_hp.T + b_hp ----
            # two psum half-contractions so the second whp DMA half overlaps
            # the first half's matmuls
            ps_h = psznp.tile([P, 8, BS], F32, tag="gzn")
            for half in range(2):
                for ko in range(KO):
                    for kf in range(8 * half, 8 * half + 8):
                        nc.tensor.matmul(
                            ps_h[:, 4 * half + ko, :],
                            whp_sb[:, kf, ko * P:(ko + 1) * P],
                            featT_sb[:, kf, :],
                            start=(kf % 8 == 0), stop=(kf % 8 == 7),
                        )
            h0s = const.tile([P, KO, BS], F32, tag="h0s")
            nc.vector.tensor_add(h0s[:], ps_h[:, 0:4, :], ps_h[:, 4:8, :])
            h0T = const.tile([P, KO, BS], F32, tag="h0T")
            for ko in range(KO):
                nc.scalar.activation(
                    h0T[:, ko, :], h0s[:, ko, :], AF.Identity,
                    bias=bhp_sb[:, ko, None], scale=1.0,
                )
            h0h = const.tile([P, KO, BS], BF16, tag="h0h")
            nc.scalar.mul(h0h[:], h0T[:], 0.5)
            h0b = const.tile([P, KO, BS], BF16, tag="h0b")
            nc.scalar.copy(h0b[:], h0T[:])

            # ---- gate constants in the gates layout (bf16 SBUF) ----
            # rz rows: g0 = W_hh@h0 + b_hh + b_ih
            # n rows:  g0 = 0.5*(W_hh@h0 + b_hh) + b_ih   (E_n form)
            # Each step the Pool engine preloads these into the gates PSUM,
            # replacing twelve per-step PE matmuls.
            ps_rz = psznp.tile([P, 8, BS], F32, tag="gzn")
            for m in range(8):
                for k in range(KO):
                    nc.tensor.matmul(
                        ps_rz[:, m, :],
                        whh_sb[:, k, m * P:(m + 1) * P],
                        h0b[:, k, :],
                        start=(k == 0), stop=(k == KO - 1),
                    )
            ps_n = psrp.tile([P, 4, BS], F32, tag="gr")
            for m in range(4):
                for k in range(KO):
                    nc.tensor.matmul(
                        ps_n[:, m, :],
                        whh_sb[:, k, (m + 8) * P:(m + 9) * P],
                        h0b[:, k, :],
                        start=(k == 0), stop=(k == KO - 1),
                    )
            g0_sb = const.tile([P, GM, BS], BF16, tag="g0sb")
            nc.vector.tensor_add(
                g0_sb[:, 0:4, :], ps_rz[:, 0:4, :],
                bsrz_sb[:, 0:4, None].to_broadcast((P, 4, BS)),
            )
            nc.vector.tensor_add(
                g0_sb[:, 4:8, :], ps_rz[:, 4:8, :],
                bsrz_sb[:, 4:8, None].to_broadcast((P, 4, BS)),
            )
            nc.vector.scalar_tensor_tensor(
                g0_sb[:, 8:12, :], ps_n[:], 0.5,
                bmixn_sb[:, :, None].to_broadcast((P, KO, BS)),
                ALU.mult, ALU.add,
            )
            hn2 = const.tile([P, KO, BS], BF16, tag="hn2")
            nc.vector.scalar_tensor_tensor(
                hn2[:], ps_n[:], 0.5,
                bhhnh_sb[:, :, None].to_broadcast((P, KO, BS)),
                ALU.mult, ALU.add,
            )

            # ---- per-granule projection state ----
            stage_tiles = {}

            def emit_proj_mm(g, u):
                if u == 0:
                    stage_tiles[g] = stagep.tile(
                        [P, VPAD], FP16, tag="stage", name=f"stage{g}"
                    )
                pp = psp.tile([P, VC], F32, tag="pp", name=f"pp{g}_{u}")
                for k in range(KO):
                    nc.tensor.matmul(
                        pp[:],
                        resT[:, k, g * GR:(g + 1) * GR, :],
                        wout[:, k, u * VC:(u + 1) * VC],
                        start=(k == 0), stop=(k == KO - 1),
                    )
                return pp

            def emit_proj_tail(g, u, pp):
                # Pool engine is otherwise idle; keeping the PSUM->fp16 drains
                # off Act/DVE protects the serial GRU chain from head-of-line
                # blocking in those engine queues.
                st = stage_tiles[g]
                nc.gpsimd.tensor_scalar_add(st[:, u * VC:(u + 1) * VC], pp[:], 0.0)
                # two half DMAs per granule so the first half's store overlaps
                # the second half's matmuls/converts
                half = NVC // 2 * VC
                if u == NVC // 2 - 1:
                    nc.sync.dma_start(
                        OUT[g * P:(g + 1) * P, 0:half], st[:, 0:half]
                    )
                elif u == NVC - 1:
                    nc.sync.dma_start(
                        OUT[g * P:(g + 1) * P, half:VPAD], st[:, half:VPAD]
                    )
                    del stage_tiles[g]

            # ---- GRU steps ----
            for t in range(STEPS):
                psr = psrp.tile([P, 4, BS], F32, tag="gr")
                pszn = psznp.tile([P, 8, BS], F32, tag="gzn")
                # Pool preloads the constant gate terms (no h dependency, so
                # this runs during the previous step's chain)
                nc.gpsimd.tensor_scalar_add(psr[:], g0_sb[:, 0:4, :], 0.0)
                nc.gpsimd.tensor_scalar_add(pszn[:], g0_sb[:, 4:12, :], 0.0)
                for m in range(GM):
                    dst = psr[:, m, :] if m < 4 else pszn[:, m - 4, :]
                    for k in range(KO):
                        rhs = x0_sb[:, k, :] if t == 0 else resT[:, k, t - 1, :]
                        nc.tensor.matmul(
                            dst, wih[:, k, m, :], rhs,
                            start=False, stop=(k == KO - 1),
                            skip_group_check=True,
                        )
                # projection units (granule g = t//GR - 1) interleave here to
                # fill the PE stream while the elementwise chain runs
                pps = []
                g = t // GR - 1
                if g >= 0:
                    for u in range(UPS * (t % GR), UPS * (t % GR) + UPS):
                        pps.append((g, u, emit_proj_mm(g, u)))

                tr = sp.tile([P, KO, BS], BF16, tag="tr")
                nc.scalar.activation(tr[:], psr[:], AF.Tanh, scale=0.5)
                tz = sp.tile([P, KO, BS], BF16, tag="tz")
                nc.scalar.activation(tz[:], pszn[:, 0:4, :], AF.Tanh, scale=0.5)
                a = sp.tile([P, KO, BS], BF16, tag="a")
                nc.vector.tensor_mul(a[:], tr[:], hn2[:])
                # Pool drains the n-gate psum so DVE's add stays all-bf16
                pn = sp.tile([P, KO, BS], BF16, tag="pn")
                nc.gpsimd.tensor_scalar_add(pn[:], pszn[:, 4:8, :], 0.0)
                sn = sp.tile([P, KO, BS], BF16, tag="sn")
                nc.vector.tensor_add(sn[:], pn[:], a[:])
                n_ = sp.tile([P, KO, BS], BF16, tag="n")
                nc.scalar.activation(n_[:], sn[:], AF.Tanh, scale=1.0)
                # d = 0.5 - 0.5*tz ; c1 = h0h*(1+tz) = h0h + h0h*tz
                d = sp.tile([P, KO, BS], BF16, tag="d")
                nc.vector.tensor_scalar(d[:], tz[:], -0.5, 0.5, ALU.mult, ALU.add)
                u_ = sp.tile([P, KO, BS], BF16, tag="u")
                nc.vector.tensor_mul(u_[:], tz[:], h0h[:])
                c1 = sp.tile([P, KO, BS], BF16, tag="c1")
                nc.vector.tensor_add(c1[:], u_[:], h0h[:])
                e = sp.tile([P, KO, BS], BF16, tag="e")
                nc.vector.tensor_mul(e[:], n_[:], d[:])
                # h' = e + c1, written straight into the res history
                nc.vector.tensor_add(resT[:, :, t, :], e[:], c1[:])

                for g, u, pp in pps:
                    emit_proj_tail(g, u, pp)

            # ---- drain the last granule's projection ----
            g = NGRAN - 1
            for u in range(NVC):
                pp = emit_proj_mm(g, u)
                emit_proj_tail(g, u, pp)

    nc.compile()
    return nc


def _shard_inputs(feat, W_hp, b_hp, W_ih, W_hh, b_ih, b_hh, embed, W_out, b_out):
    bf = ml_dtypes.bfloat16
    feat = np.asarray(feat)
    WhpT = np.ascontiguousarray(np.asarray(W_hp).T, dtype=np.float16)
    WhhT = np.ascontiguousarray(np.asarray(W_hh).T).astype(bf)
    WihT = np.ascontiguousarray(np.asarray(W_ih).T).astype(bf)
    x0T = np.ascontiguousarray(
        np.repeat(np.asarray(embed)[SOS][:, None], BS, axis=1)
    ).astype(bf)
    b_ih = np.asarray(b_ih, np.float32)
    b_hh = np.asarray(b_hh, np.float32)
    bsum_rz = (b_hh + b_ih)[:2 * HID].copy()
    bmix_n = (0.5 * b_hh + b_ih)[2 * HID:].copy()
    bhhn_half = (0.5 * b_hh)[2 * HID:].copy()
    Wo = np.zeros((NVQ * VPAD, HID), np.float32)
    Wo[:VOCAB] = np.asarray(W_out)
    common = dict(
        WhpT=WhpT, WhhT=WhhT, WihT=WihT, x0T=x0T,
        b_hp=np.asarray(b_hp, np.float32),
        bsum_rz=bsum_rz, bmix_n=bmix_n, bhhn_half=bhhn_half,
    )
    featT_halves = [
        np.ascontiguousarray(feat[hb * BS:(hb + 1) * BS].T, dtype=np.float32)
        for hb in range(2)
    ]
    woutT_quarters = [
        np.ascontiguousarray(Wo[vq * VPAD:(vq + 1) * VPAD].T).astype(bf)
        for vq in range(NVQ)
    ]
    in_maps = []
    for c in range(NCORES):
        hb, vq = divmod(c, NVQ)
        m = dict(common)
        m["featT"] = featT_halves[hb]
        m["WoutT"] = woutT_quarters[vq]
        in_maps.append(m)
    return in_maps


def kernel(**inputs):
    global LAST_RESULTS
    args = {k: np.asarray(v) for k, v in inputs.items()}
    in_maps = _shard_inputs(
        args["feat"], args["W_hp"], args["b_hp"], args["W_ih"], args["W_hh"],
        args["b_ih"], args["b_hh"], args["embed"], args["W_out"], args["b_out"],
    )
    nc = build()
    res = run_bass_kernel_spmd(nc, in_maps, core_ids=list(range(NCORES)))
    LAST_RESULTS = res
    full = np.empty((BATCH, VOCAB, STEPS), np.float32)
    for c in range(NCORES):
        hb, vq = divmod(c, NVQ)
        v0 = vq * VPAD
        nv = min(VPAD, VOCAB - v0)
        if nv <= 0:
            continue
        # OUT is [(T*BS), VPAD] fp16, row t*BS + b
        o = np.asarray(res.results[c]["OUT"], dtype=np.float32)
        o = o.reshape(STEPS, BS, VPAD)
        full[hb * BS:(hb + 1) * BS, v0:v0 + nv, :] = (
            o[:, :, :nv].transpose(1, 2, 0)
        )
    b_out = np.asarray(args["b_out"], np.float32)
    if np.any(b_out):
        full += b_out[None, :, None]
    return np.ascontiguousarray(full, dtype=np.float32)
